# revision 61
# baseline (speedup 1.0000x reference)
"""Trainium2 Bass kernel for nn_BaselineModel_74509092651544 (CLRS-style MPNN).

Strategy
--------
Data-parallel over graphs: 32 graphs -> 8 cores x 4 graphs.  The dense
[B,N,N,H] message tensor of the reference is never materialized: only the
~62k unique (graph,src,dst) edge slots survive the masked max, so the
message MLP runs on a padded uniform-K slot layout (~8.5x less compute).

Per core, everything lives in SBUF feature-major [H=128, cols]:
  * node embeddings via fp8 DoubleRow one-hot matmuls (hi+res two-term
    split keeps ~bf16 accuracy at fp8 double rate),
  * m1[src]+m2[dst]+edge_fts@We accumulated in PSUM via 2 fp8 DoubleRow
    matmuls per 256-slot chunk: (m1,m2)@(Gsrc,Gdst) pairs the two gather
    matrices; (bw0,bw1)@(soh0,soh1) splits the 48-dim bond one-hot in two
    24-dim halves.  Gather/one-hot matrices are host-built integers.
  * the 2-layer message MLP as bf16 matmuls at up to N=512,
  * masked max over senders as one DVE segmented reduce per uniform-K tile,
  * PSUM->SBUF relu/copy traffic load-balanced across ACT/DVE/Pool engines,
  * LayerNorm via PE transposes + DVE bn_stats; ln scale/bias folded into
    the next layer's weights host-side (exact algebra),
  * graph pooling + prediction MLP on-device; output is [OUT, 4] per core.

Host work is integer indexing / relayout plus parameter-only folds
(bias sums, ln folding, bond_emb@We).  All data-dependent float math
happens on device.
"""

import sys
import numpy as np

sys.path.insert(0, "/opt/trn_rl_repo")

B, N, H, L, E, OUT = 32, 128, 128, 3, 65536, 128
M = 8                 # NeuronCores
BL = B // M           # graphs per core
NEG = -1e9
EPS = 1e-5
AV, BV = 128, 16

MT_HIRES = False    # two-term fp8 m1/m2 split (extra accuracy, ~5us slower)
_CACHE = {}


# --------------------------------------------------------------------------
# Host preprocessing: integer indexing / relayout.
# --------------------------------------------------------------------------

def _prep(inputs):
    import ml_dtypes
    FP8 = ml_dtypes.float8_e4m3fn

    x = np.asarray(inputs["x"]).astype(np.int64)            # [B*N, 9]
    ea = np.asarray(inputs["edge_attr"]).astype(np.int64)   # [E, 3]
    ei = np.asarray(inputs["edge_index"]).astype(np.int64)  # [2, E]

    g = ei[0] // N
    s = ei[0] % N
    d = ei[1] % N
    key = (g * N + s) * N + d
    uniq, inv = np.unique(key, return_inverse=True)
    US = uniq.size
    ug = uniq // (N * N)
    us = (uniq // N) % N
    ud = uniq % N

    # bond one-hot counts per unique slot  [US, 48]
    oh48 = np.zeros((US, 48), np.float32)
    for c in range(3):
        np.add.at(oh48, (inv, ea[:, c] + 16 * c), 1.0)
    assert float(oh48.max()) <= 16.0

    # unique in-degree per (graph, receiver)
    deg = np.zeros((B, N), np.int64)
    np.add.at(deg, (ug, ud), 1)

    # receiver relabeling: position p holds the p-th highest-degree receiver
    rho = np.argsort(-deg, axis=1, kind="stable")        # [B, N] pos -> orig
    rho_inv = np.argsort(rho, axis=1)                    # orig -> pos
    degS = -np.sort(-deg, axis=1)                        # [B, N] desc
    Kp = np.maximum(degS.max(axis=0), 1)                 # [N]

    # uniform-K tiles: positions p0..p0+R-1 all padded to K = Kp[p0]
    tiles = []
    p = 0
    while p < N:
        K = int(Kp[p])
        R = min(512 // K, N - p)
        tiles.append((p, R, K))
        p += R
    n_tiles = len(tiles)
    S_graph = 512 * n_tiles
    S_core = BL * S_graph
    tile_w = [R * K for (_, R, K) in tiles]

    tile_of_pos = np.zeros(N, np.int64)
    K_of_pos = np.zeros(N, np.int64)
    p0_of_pos = np.zeros(N, np.int64)
    for t, (p0, R, K) in enumerate(tiles):
        tile_of_pos[p0:p0 + R] = t
        K_of_pos[p0:p0 + R] = K
        p0_of_pos[p0:p0 + R] = p0

    # slots ordered by (g, d, s): contiguous per receiver
    order = np.lexsort((us, ud, ug))
    og, od, osl = ug[order], ud[order], order
    osrc = us[order]
    recv_id = og * N + od
    first = np.concatenate([[0], np.flatnonzero(np.diff(recv_id)) + 1])
    k_rank = np.arange(len(og)) - first[np.searchsorted(recv_id[first], recv_id)]

    pos = rho_inv[og, od]

    # padding: receivers with deg < K duplicate their first slot
    fg, fd = og[first], od[first]
    fpos = rho_inv[fg, fd]
    fdeg = deg[fg, fd]
    fK = K_of_pos[fpos]
    padc = (fK - fdeg).astype(np.int64)
    assert (padc >= 0).all()
    rep = np.repeat(np.arange(len(first)), padc)
    kpad = np.arange(len(rep)) - np.repeat(
        np.concatenate([[0], np.cumsum(padc)[:-1]]), padc
    ) + np.repeat(fdeg, padc)
    pg = fg[rep]
    pos_p = fpos[rep]
    src_p = osrc[first][rep]
    slot_p = osl[first][rep]

    a_g = np.concatenate([og, pg])
    a_pos = np.concatenate([pos, pos_p])
    a_k = np.concatenate([k_rank, kpad])
    a_slot = np.concatenate([osl, slot_p])
    a_srcnew = np.concatenate([rho_inv[og, osrc], rho_inv[pg, src_p]])

    # slot -> chunked gsd/soh2 columns
    a_t = tile_of_pos[a_pos]
    a_w = np.asarray(tile_w)[a_t]
    a_x = (a_pos - p0_of_pos[a_pos]) * K_of_pos[a_pos] + a_k
    wc0 = np.minimum(a_w, 256)
    a_c = (a_x >= wc0).astype(np.int64)          # chunk index (0 or 1)
    a_xi = a_x - a_c * 256
    a_wcc = np.where(a_c == 0, wc0, a_w - 256)
    gl = a_g % BL
    core = a_g // BL
    gbase = 2 * (gl * S_graph + a_t * 512) + a_c * 512
    col_src = gbase + a_xi
    col_dst = gbase + a_wcc + a_xi

    flat_src = core * (2 * S_core) + col_src
    flat_dst = core * (2 * S_core) + col_dst

    Gsd = np.zeros((M * 2 * S_core, 128), np.float32)
    Gsd[flat_src, a_srcnew] = 1.0
    a_dstpos = a_pos
    Gsd[flat_dst, a_dstpos] = 1.0
    Gsd = np.ascontiguousarray(
        Gsd.reshape(M, 2 * S_core, 128).transpose(0, 2, 1)).astype(FP8)

    # soh2: rows 0:24 hold bond-dims 0:24 at src-half columns and bond-dims
    # 24:48 at dst-half columns; values are counts/16 (exact in fp8).
    SOH2 = np.zeros((M * 2 * S_core, 24), np.float32)
    SOH2[flat_src] = oh48[a_slot, 0:24] * (1.0 / 16.0)
    SOH2[flat_dst] = oh48[a_slot, 24:48] * (1.0 / 16.0)
    SOH2 = np.ascontiguousarray(
        SOH2.reshape(M, 2 * S_core, 24).transpose(0, 2, 1)).astype(FP8)

    # atom one-hot per core: [9, 128, BL*N] in relabeled node order
    gg = np.repeat(np.arange(B), N)
    pp = np.tile(np.arange(N), B)
    orig = gg * N + rho[gg, pp]                    # [B*N] column -> orig node
    XOH = np.zeros((M, 9, AV, BL * N), np.float32)
    mcol = np.tile(np.arange(BL * N), M)
    mcore = np.repeat(np.arange(M), BL * N)
    for c in range(9):
        XOH[mcore, c, x[orig, c], mcol] = 1.0
    # [M, AV, 9*BL*N] c-major columns: one resident DMA instead of a
    # double-buffered stream (the 2-deep ring serialized the fill phase)
    XOH = np.ascontiguousarray(XOH.transpose(0, 2, 1, 3).reshape(
        M, AV, 9 * BL * N)).astype(FP8)

    # empty receivers (deg==0) -> NEG mask path
    empt = (deg == 0)
    has_empty = bool(empt.any())
    maskrow = np.ones((M, BL * N), np.float32)
    negrow = np.zeros((M, BL * N), np.float32)
    if has_empty:
        eg, en = np.nonzero(empt)
        epos = rho_inv[eg, en]
        maskrow[eg // BL, (eg % BL) * N + epos] = 0.0
        negrow[eg // BL, (eg % BL) * N + epos] = NEG

    struct = dict(
        S_graph=S_graph, S_core=S_core, n_tiles=n_tiles,
        tiles=tuple(tiles), tile_w=tuple(tile_w), has_empty=has_empty,
    )
    percore = dict(gsd=Gsd, soh2=SOH2, xoh=XOH,
                   maskrow=maskrow, negrow=negrow)
    return struct, percore


def _weight_arrays(inputs):
    import ml_dtypes
    f32 = np.float32
    BF16 = ml_dtypes.bfloat16
    FP8 = ml_dtypes.float8_e4m3fn
    A = {}

    Wm1 = np.asarray(inputs["Wm1"], f32)
    Wm2 = np.asarray(inputs["Wm2"], f32)
    We = np.asarray(inputs["We"], f32)
    Wo1 = np.asarray(inputs["Wo1"], f32)
    Wo2 = np.asarray(inputs["Wo2"], f32)
    Wp1 = np.asarray(inputs["Wp1"], f32)
    Wp2 = np.asarray(inputs["Wp2"], f32)
    ln_s = np.asarray(inputs["ln_s"], f32)
    ln_b = np.asarray(inputs["ln_b"], f32)
    bond = np.asarray(inputs["bond_emb"], f32).reshape(48, H)
    atom = np.asarray(inputs["atom_emb"], f32)

    # ln fold: hidden entering layer l is hid_used = hnorm*ln_s[l-1]+ln_b[l-1]
    # (exact algebra; layer 0 uses raw zeros -> identity fold)
    lnS = np.concatenate([np.ones((1, H), f32), ln_s[:L - 1]], 0)   # [L, H]
    lnB = np.concatenate([np.zeros((1, H), f32), ln_b[:L - 1]], 0)

    # weight blob (bf16, one DMA): per-layer stationary/moving tiles
    cols = []
    wmap = {}

    def add(name, arr):
        wmap[name] = (sum(c.shape[1] for c in cols), arr.shape[1])
        cols.append(np.asarray(arr, f32))

    add("idn", np.eye(128, dtype=f32))
    add("ones", np.ones((128, 1), f32))
    for l in range(L):
        m1t, m1b = Wm1[l, 0:128], Wm1[l, 128:256] * lnS[l][:, None]
        m2t, m2b = Wm2[l, 0:128], Wm2[l, 128:256] * lnS[l][:, None]
        add(f"m12_{l}_0", np.concatenate([m1t, m2t], 1))
        add(f"m12_{l}_1", np.concatenate([m1b, m2b], 1))
        add(f"Wp1_{l}", Wp1[l])
        add(f"Wp2_{l}", Wp2[l])
        add(f"Wo1_{l}_0", Wo1[l, 0:128])
        add(f"Wo1_{l}_1", Wo1[l, 128:256] * lnS[l][:, None])
        add(f"Wo2_{l}", Wo2[l])
    A["wblob"] = np.ascontiguousarray(np.concatenate(cols, 1)).astype(BF16)
    A["_wmap"] = wmap

    # head weights stay f32r: their error would hit the output directly
    # (plus one f32r ones-column for the layernorm sum-of-squares matmul)
    Wh1 = np.asarray(inputs["Wh1"], f32)
    A["whb"] = np.ascontiguousarray(np.concatenate(
        [ln_s[L - 1][:, None] * Wh1, np.asarray(inputs["Wh2"], f32),
         np.ones((H, 1), f32)], 1))

    # atom embeddings: fp8 hi/res pairs of 16*atom_emb [AV, 9*256]
    # (x16 keeps the residual term out of fp8 subnormals; nf copy scales back)
    at = atom.transpose(1, 0, 2) * 16.0                # [AV, 9, H]
    hi = at.astype(FP8)
    res = (at - hi.astype(f32)).astype(FP8)
    pair = np.concatenate([hi[:, :, None, :], res[:, :, None, :]], 2)
    A["atomb"] = np.ascontiguousarray(pair.reshape(AV, 9 * 2 * H))

    # bond tables: bw2[l] = [We[l]^T rows for bond dims 0:24 | 24:48] * 16
    bw = np.einsum("bh,lhf->lbf", bond, We)            # [L, 48, H]
    bw2 = np.concatenate([bw[:, 0:24], bw[:, 24:48]], 2) * 16.0  # [L,24,2H]
    A["bw2"] = np.ascontiguousarray(
        bw2.transpose(1, 0, 2).reshape(24, L * 2 * H)).astype(FP8)

    # bias columns [128, 3L+3]: bias_pre(L), bp1(L), bias_h(L), eps, bh1, bh2h
    bm1 = np.asarray(inputs["bm1"], f32)
    bm2 = np.asarray(inputs["bm2"], f32)
    be = np.asarray(inputs["be"], f32)
    bg = np.asarray(inputs["bg"], f32)
    bp1 = np.asarray(inputs["bp1"], f32)
    bp2 = np.asarray(inputs["bp2"], f32)
    bo1 = np.asarray(inputs["bo1"], f32)
    bo2 = np.asarray(inputs["bo2"], f32)
    bias_pre = (bm1 + bm2 + be + bg
                + np.einsum("lh,lhf->lf", lnB,
                            Wm1[:, 128:256] + Wm2[:, 128:256]))     # [L, H]
    bias_h = (bo1 + bo2 + np.einsum("lh,lhf->lf", bp2, Wo2)
              + np.einsum("lh,lhf->lf", lnB, Wo1[:, 128:256]))      # [L, H]
    bh1_eff = np.asarray(inputs["bh1"], f32) + ln_b[L - 1] @ Wh1

    bc = np.zeros((H, 3 * L + 3), f32)
    bc[:, 0:L] = bias_pre.T
    bc[:, L:2 * L] = bp1.T
    bc[:, 2 * L:3 * L] = bias_h.T
    bc[:, 3 * L] = EPS
    bc[:, 3 * L + 1] = bh1_eff
    bc[:, 3 * L + 2] = np.asarray(inputs["bh2"], f32)[:H]
    A["bias_cols"] = bc
    A["bh2_full"] = np.ascontiguousarray(
        np.asarray(inputs["bh2"], f32).reshape(OUT, 1))
    return A


# --------------------------------------------------------------------------
# Bass program.
# --------------------------------------------------------------------------

def _build_program(struct, wmap, wcols):
    import concourse.bacc as bacc
    import concourse.mybir as mybir
    import concourse.tile as tile

    F32 = mybir.dt.float32
    BF16 = mybir.dt.bfloat16
    FP8 = mybir.dt.float8e4

    S_core = struct["S_core"]

    nc = bacc.Bacc("TRN2", target_bir_lowering=False, debug=False)

    d = {}
    d["d_gsd"] = nc.dram_tensor("gsd", [128, 2 * S_core], FP8,
                                kind="ExternalInput")
    d["d_soh2"] = nc.dram_tensor("soh2", [24, 2 * S_core], FP8,
                                 kind="ExternalInput")
    d["d_axoh"] = nc.dram_tensor("axoh", [AV, 9 * 2 * H + 9 * BL * N], FP8,
                                 kind="ExternalInput")
    d["d_bw2"] = nc.dram_tensor("bw2", [24, L * 2 * H], FP8,
                                kind="ExternalInput")
    d["d_wblob"] = nc.dram_tensor("wblob", [128, wcols], BF16,
                                  kind="ExternalInput")
    d["d_whb"] = nc.dram_tensor("whb", [128, 2 * H + 1], mybir.dt.float32r,
                                kind="ExternalInput")
    d["d_bc"] = nc.dram_tensor("bias_cols", [H, 3 * L + 3], F32,
                               kind="ExternalInput")
    d["d_bh2"] = nc.dram_tensor("bh2_full", [OUT, 1], F32,
                                kind="ExternalInput")
    d["d_mask"] = nc.dram_tensor("maskrow", [1, BL * N], F32,
                                 kind="ExternalInput")
    d["d_neg"] = nc.dram_tensor("negrow", [1, BL * N], F32,
                                kind="ExternalInput")
    d["d_out"] = nc.dram_tensor("out", [OUT, BL], F32, kind="ExternalOutput")

    with tile.TileContext(nc) as tc:
        _emit(tc, nc, d, struct, wmap, mybir)
    nc.compile()
    return nc


def _emit(tc, nc, d, struct, wmap, mybir):
    import contextlib
    ctx = contextlib.ExitStack()
    F32 = mybir.dt.float32
    BF16 = mybir.dt.bfloat16
    FP8 = mybir.dt.float8e4
    AF = mybir.ActivationFunctionType
    ALU = mybir.AluOpType
    AX = mybir.AxisListType
    DR = mybir.MatmulPerfMode.DoubleRow

    S_graph = struct["S_graph"]
    S_core = struct["S_core"]
    n_tiles = struct["n_tiles"]
    tiles = struct["tiles"]
    tile_w = struct["tile_w"]
    has_empty = struct["has_empty"]

    pG = ctx.enter_context(tc.tile_pool(name="pG", bufs=1))
    pW = ctx.enter_context(tc.tile_pool(name="pW", bufs=1))
    pAct = ctx.enter_context(tc.tile_pool(name="pAct", bufs=6))
    pNM = ctx.enter_context(tc.tile_pool(name="pNM", bufs=1))
    pMB = ctx.enter_context(tc.tile_pool(name="pMB", bufs=2))
    pLN = ctx.enter_context(tc.tile_pool(name="pLN", bufs=2))
    pIn = ctx.enter_context(tc.tile_pool(name="pIn", bufs=2))
    ps_pre = ctx.enter_context(tc.tile_pool(name="ps_pre", bufs=2, space="PSUM"))
    ps_p1 = ctx.enter_context(tc.tile_pool(name="ps_p1", bufs=2, space="PSUM"))
    ps_p2 = ctx.enter_context(tc.tile_pool(name="ps_p2", bufs=2, space="PSUM"))
    ps_misc = ctx.enter_context(tc.tile_pool(name="ps_misc", bufs=2, space="PSUM"))

    def mps(name, dt=F32):
        return ps_misc.tile([128, 512], dt, name=name, tag="mps")

    # ---- engine load balancer for PSUM->SBUF relu/copy traffic (ns units)
    eng_load = {"act": 0.0, "dve": 0.0, "pool": 0.0}
    ECOST = {"act": lambda w: w * 0.833 + 170.0,
             "dve": lambda w: w * 1.0417 + 145.0,
             "pool": lambda w: w * 1.39 + 131.0}

    def assign(w, only=None, engines=("act", "dve")):
        # GPSIMD/Pool cannot access PSUM on TRN2, so PSUM-draining work is
        # restricted to ACT/DVE; SBUF-only ops may pass engines with "pool"
        if only is not None:
            e = only
        else:
            e = min(engines, key=lambda k: eng_load[k] + ECOST[k](w))
        eng_load[e] += ECOST[e](w)
        return e

    def emit_relu(out, in_, w, bias, only=None):
        e = assign(w, only)
        if e == "act":
            nc.scalar.activation(out, in_, AF.Relu, bias=bias)
        elif e == "dve":
            nc.vector.tensor_scalar(out, in_, bias, 0.0,
                                    op0=ALU.add, op1=ALU.max)
        else:
            nc.gpsimd.tensor_scalar(out, in_, bias, 0.0,
                                    op0=ALU.add, op1=ALU.max)

    def emit_copy(out, in_, w, only=None):
        e = assign(w, only)
        if e == "act":
            nc.scalar.activation(out, in_, AF.Copy)
        elif e == "dve":
            nc.vector.tensor_copy(out, in_)
        else:
            nc.gpsimd.tensor_copy(out, in_)

    def charge_ns(e, ns):
        eng_load[e] += ns

    # ---- resident loads (small/early-needed tensors first)
    bc_sb = pW.tile([H, 3 * L + 3], F32, name="bc_sb")
    nc.sync.dma_start(bc_sb[:], d["d_bc"].ap())
    wcols = sum(w for (_, w) in wmap.values())
    wblob_sb = pW.tile([128, wcols], BF16, name="wblob_sb")
    nc.sync.dma_start(wblob_sb[:], d["d_wblob"].ap())
    atomb_sb = pW.tile([AV, 9 * 2 * H], FP8, name="atomb_sb")
    nc.sync.dma_start(atomb_sb[:], d["d_atomb"].ap())
    bw2_sb = pW.tile([24, L * 2 * H], FP8, name="bw2_sb")
    nc.sync.dma_start(bw2_sb[:], d["d_bw2"].ap())
    bh2_sb = pW.tile([OUT, 1], F32, name="bh2_sb")
    nc.sync.dma_start(bh2_sb[:], d["d_bh2"].ap())
    F32R = mybir.dt.float32r
    whb_sb = pW.tile([128, 2 * H + 1], F32R, name="whb_sb")
    nc.sync.dma_start(whb_sb[:], d["d_whb"].ap())

    def W(name):
        off, w = wmap[name]
        return wblob_sb[:, off:off + w]

    idn_sb = W("idn")

    if has_empty:
        mrow_sb = pW.tile([1, BL * N], F32, name="mrow_sb")
        nc.sync.dma_start(mrow_sb[:], d["d_mask"].ap())
        nrow_sb = pW.tile([1, BL * N], F32, name="nrow_sb")
        nc.sync.dma_start(nrow_sb[:], d["d_neg"].ap())
        mask_bc = pW.tile([128, BL * N], F32, name="mask_bc")
        nc.gpsimd.partition_broadcast(mask_bc[:], mrow_sb[:])
        neg_bc = pW.tile([128, BL * N], F32, name="neg_bc")
        nc.gpsimd.partition_broadcast(neg_bc[:], nrow_sb[:])

    gsd_sb = pG.tile([128, 2 * S_core], FP8, name="gsd_sb")
    soh2_sb = pG.tile([24, 2 * S_core], FP8, name="soh2_sb")

    # ---- node features via fp8 DoubleRow (hi,res) pairs; hidden0 = 0
    xoh_sb = axoh_sb[:, 9 * 2 * H:]
    nf_ps = mps("nf_ps")
    nf = pNM.tile([128, BL * N], BF16, name="nf")
    for half in range(2):
        # half-major order with a split copy: the first m1/m2 block only
        # needs graph 0's node columns, so layer 0 starts ~2us earlier
        for c in range(9):
            st = atomb_sb[:, c * 2 * H:(c + 1) * 2 * H].rearrange(
                "p (two m) -> p two m", two=2)
            co = c * BL * N + half * 256
            mv = xoh_sb[:, co:co + 256].rearrange(
                "p (one n) -> p one n", one=1).broadcast_to([AV, 2, 256])
            nc.tensor.matmul(nf_ps[:, half * 256:(half + 1) * 256], st, mv,
                             start=(c == 0), stop=(c == 8), perf_mode=DR)
        nc.scalar.activation(nf[:, half * 256:(half + 1) * 256],
                             nf_ps[:, half * 256:(half + 1) * 256],
                             AF.Copy, scale=1.0 / 16.0)
    hid0 = pNM.tile([128, BL * N], BF16, name="hid0", tag="hid0")
    nc.vector.memset(hid0[:], 0.0)
    # prime the ACT Sqrt table while the engine is idle (a mid-run table
    # load costs 1283ns on the saturated ACT engine)
    warm = pLN.tile([128, 1], F32, name="warm", tag="warm")
    nc.scalar.activation(warm[:], bc_sb[:, 3 * L:3 * L + 1], AF.Sqrt)

    for gg in range(BL):
        sl = slice(2 * gg * S_graph, 2 * (gg + 1) * S_graph)
        nc.sync.dma_start(gsd_sb[:, sl], d["d_gsd"].ap()[:, sl])
        nc.sync.dma_start(soh2_sb[:, sl], d["d_soh2"].ap()[:, sl])

    hid_prev = hid0

    # ----------------------------------------------------------------
    # Globally software-pipelined job loop: jobs = (layer, graph, tile).
    # Stage lags keep the in-order PE stream busy: at step i the PE runs
    # DR(i), p1(i-1), p2(i-2) while ACT/DVE/Pool run the relus of earlier
    # jobs, so no engine ever waits head-of-line on a cross-engine hop.
    # ----------------------------------------------------------------
    jobs = [(l, gg, t) for l in range(L) for gg in range(BL)
            for t in range(n_tiles)]
    J = len(jobs)
    st8 = {}          # per-job state: pre/p1/p2/msgs tiles
    layer_st = {}     # per-layer tiles (msgs_max, h_fm, hid)
    graph_st = {}     # per-(layer,graph) tiles (mt, hn, ...)

    def layer_state(l):
        if l in layer_st:
            return layer_st[l]
        s = dict(
            msgs_max=pLN.tile([128, BL * N], BF16, name=f"msgs_max{l}",
                              tag=f"msgs_max{l % 2}", bufs=1),
            h_fm=pLN.tile([128, BL * N], BF16, name=f"h_fm{l}",
                          tag=f"h_fm{l % 2}", bufs=1),
            hid=(pNM.tile([128, BL * N], BF16, name=f"hid{l + 1}",
                          tag=f"hid{(l + 1) % 2}") if l < L - 1 or True
                 else None),
            bw2_l=bw2_sb[:, l * 2 * H:(l + 1) * 2 * H].rearrange(
                "p (two m) -> p two m", two=2),
        )
        layer_st[l] = s
        return s

    def emit_m12a(l, gg):
        ls = layer_state(l)
        hid_in = hid0 if l == 0 else layer_st[l - 1]["hid"]
        gsl = slice(gg * N, (gg + 1) * N)
        ps_m = mps("ps_m")
        nc.tensor.matmul(ps_m[:, 0:2 * H], nf[:, gsl],
                         W(f"m12_{l}_0"), start=True, stop=(l == 0))
        if l > 0:
            nc.tensor.matmul(ps_m[:, 0:2 * H], hid_in[:, gsl],
                             W(f"m12_{l}_1"), start=False, stop=True)
        # optional hi/res two-term fp8 split of (m1|m2) (~bf16 accuracy)
        mt = pMB.tile([128, 4 * H], FP8, name=f"mt{gg}", tag=f"mt{gg}",
                      bufs=2)
        emit_copy(mt[:, 0:2 * H], ps_m[:, 0:2 * H], 2 * H)
        graph_st[(l, gg)] = dict(
            ps_m=ps_m, mt=mt,
            mt_hi=mt[:, 0:2 * H].rearrange("p (two m) -> p two m", two=2),
            mt_res=mt[:, 2 * H:4 * H].rearrange("p (two m) -> p two m",
                                                two=2))

    def emit_m12b(l, gg):
        if not MT_HIRES:
            return
        gs = graph_st[(l, gg)]
        mt, ps_m = gs["mt"], gs["ps_m"]
        nc.vector.tensor_tensor(mt[:, 2 * H:4 * H], ps_m[:, 0:2 * H],
                                mt[:, 0:2 * H], op=ALU.subtract)
        charge_ns("dve", 2 * H * 1.0417 + 125.0)

    def emit_dr(i):
        l, gg, t = jobs[i]
        gs = graph_st[(l, gg)]
        ls = layer_state(l)
        w = tile_w[t]
        base = 2 * (gg * S_graph + t * 512)
        pre = ps_pre.tile([128, 512], F32, name="pre")
        nchunks = 1 if w <= 256 else 2
        chunks = []
        for c in range(nchunks):
            wc = min(w, 256) if c == 0 else w - 256
            chunks.append((slice(c * 256, c * 256 + wc), base + c * 512, wc))
        # grouped by stationary operand to minimize LdWeights swaps; one
        # accumulation group per PSUM bank (start arms the whole 2KB
        # region, chunk 1's first write replaces, later terms accumulate)
        stns = (gs["mt_hi"], gs["mt_res"]) if MT_HIRES else (gs["mt_hi"],)
        for si, stn in enumerate(stns):
            for ci, (cr, go, wc) in enumerate(chunks):
                mv_g = gsd_sb[:, go:go + 2 * wc].rearrange(
                    "p (two n) -> p two n", two=2)
                nc.tensor.matmul(pre[:, cr], stn, mv_g,
                                 start=(si == 0 and ci == 0),
                                 stop=False, perf_mode=DR)
        for ci, (cr, go, wc) in enumerate(chunks):
            mv_s = soh2_sb[:, go:go + 2 * wc].rearrange(
                "p (two n) -> p two n", two=2)
            nc.tensor.matmul(pre[:, cr], ls["bw2_l"], mv_s,
                             start=False, stop=(ci == len(chunks) - 1),
                             perf_mode=DR)
        msgs1 = pAct.tile([128, 512], BF16, name="msgs1", tag="msgs1")
        emit_relu(msgs1[:, 0:w], pre[:, 0:w], w, bc_sb[:, l:l + 1])
        st8[i] = dict(msgs1=msgs1)

    def emit_p1(i):
        l, gg, t = jobs[i]
        w = tile_w[t]
        p1 = ps_p1.tile([128, 512], F32, name="p1")
        nc.tensor.matmul(p1[:, 0:w], W(f"Wp1_{l}"), st8[i]["msgs1"][:, 0:w],
                         start=True, stop=True)
        msgs2 = pAct.tile([128, 512], BF16, name="msgs2", tag="msgs2")
        emit_relu(msgs2[:, 0:w], p1[:, 0:w], w, bc_sb[:, L + l:L + l + 1])
        st8[i]["msgs2"] = msgs2

    def emit_p2(i):
        l, gg, t = jobs[i]
        ls = layer_st[l]
        w = tile_w[t]
        p2 = ps_p2.tile([128, 512], F32, name="p2")
        nc.tensor.matmul(p2[:, 0:w], W(f"Wp2_{l}"), st8[i]["msgs2"][:, 0:w],
                         start=True, stop=True)
        (p0, R, K) = tiles[t]
        nc.vector.tensor_reduce(
            ls["msgs_max"][:, gg * N + p0: gg * N + p0 + R],
            p2[:, 0:w].rearrange("p (r k) -> p r k", r=R),
            axis=AX.X, op=ALU.max)
        charge_ns("dve", w * 1.0417 + 125.0)
        del st8[i]

    def emit_h(l, gg):
        ls = layer_st[l]
        hid_in = hid0 if l == 0 else layer_st[l - 1]["hid"]
        gsl = slice(gg * N, (gg + 1) * N)
        msl = ls["msgs_max"][:, gsl]
        if has_empty:
            mm1 = pLN.tile([128, N], F32, name="mm1", tag="mm1")
            nc.vector.tensor_tensor(mm1[:], msl, mask_bc[:, gsl],
                                    op=ALU.mult)
            nc.vector.tensor_tensor(mm1[:], mm1[:], neg_bc[:, gsl],
                                    op=ALU.add)
            mm2 = pLN.tile([128, N], BF16, name="mm2", tag="mm2")
            nc.vector.tensor_copy(mm2[:], mm1[:])
            msl = mm2[:]
        h_ps = mps("h_ps")
        nc.tensor.matmul(h_ps[:, 0:N], W(f"Wo1_{l}_0"), nf[:, gsl],
                         start=True, stop=False)
        nc.tensor.matmul(h_ps[:, 0:N], W(f"Wo1_{l}_1"), hid_in[:, gsl],
                         start=False, stop=False)
        nc.tensor.matmul(h_ps[:, 0:N], W(f"Wo2_{l}"), msl,
                         start=False, stop=True)
        emit_relu(ls["h_fm"][:, gsl], h_ps[:, 0:N], N,
                  bc_sb[:, 2 * L + l:2 * L + l + 1])

    def emit_ln1a(l, gg):
        # transpose to node-major; bn_stats reads the PSUM tile directly
        ls = layer_st[l]
        gsl = slice(gg * N, (gg + 1) * N)
        tp = mps("tp_ps", BF16)
        nc.tensor.transpose(tp[:, 0:128], ls["h_fm"][:, gsl], idn_sb)
        bn6 = pLN.tile([128, 6], F32, name="bn6", tag=f"bn6_{gg}", bufs=1)
        nc.vector.bn_stats(bn6[:], tp[:, 0:128])
        charge_ns("dve", 260.0)
        if l == L - 1:
            # drain phase: free the PSUM ring slot fast so the remaining
            # ladders don't serialize behind each other
            hT = pLN.tile([128, 128], BF16, name="hT", tag=f"hT_{gg}",
                          bufs=1)
            emit_copy(hT[:], tp[:, 0:128], 128)
            graph_st[(l, gg)].update(tp=None, hT=hT, bn6=bn6)
        else:
            graph_st[(l, gg)].update(tp=tp, hT=None, bn6=bn6)

    def emit_ln1b(l, gg):
        gs = graph_st[(l, gg)]
        bn2 = pLN.tile([128, 2], F32, name="bn2", tag=f"bn2_{gg}", bufs=1)
        nc.vector.bn_aggr(bn2[:], gs["bn6"][:])
        std = pLN.tile([128, 1], F32, name="std", tag=f"std_{gg}", bufs=1)
        nc.scalar.activation(std[:], bn2[:, 1:2], AF.Sqrt,
                             bias=bc_sb[:, 3 * L:3 * L + 1])
        gs.update(bn2=bn2, std=std)

    def emit_ln1c(l, gg):
        # normalize on DVE straight from the bf16 PSUM transpose (2-byte
        # packed operands hit the 2x DVE mode); rstd via DVE reciprocal
        gs = graph_st[(l, gg)]
        rstd = pLN.tile([128, 1], F32, name="rstd", tag=f"rstd_{gg}",
                        bufs=1)
        nc.vector.reciprocal(rstd[:], gs["std"][:])
        hn = pLN.tile([128, 128], BF16, name="hn", tag=f"hn_{gg}", bufs=1)
        src_ap = gs["hT"][:] if gs["tp"] is None else gs["tp"][:, 0:128]
        nc.vector.tensor_scalar(hn[:], src_ap, gs["bn2"][:, 0:1],
                                rstd[:], op0=ALU.subtract, op1=ALU.mult)
        charge_ns("dve", 192.0)
        gs["hn"] = hn

    def emit_ln2(l, gg):
        # transpose back to feature-major hid
        ls = layer_st[l]
        gsl = slice(gg * N, (gg + 1) * N)
        tp2 = mps("tp2_ps", BF16)
        nc.tensor.transpose(tp2[:, 0:128], graph_st[(l, gg)]["hn"][:],
                            idn_sb)
        emit_copy(ls["hid"][:, gsl], tp2[:, 0:128], N)

    LAST = n_tiles - 1
    LAGP1, LAGP2 = 2, 3
    import collections as _c
    sched = _c.defaultdict(list)
    for j, (l, gg, t) in enumerate(jobs):
        if t == 0:
            sched[max(j - 4, 0)].append(("m12a", l, gg))
            sched[max(j - 3, 0)].append(("m12b", l, gg))
        if t == LAST:
            e = j
            sched[e + LAGP2].append(("h", l, gg))
            sched[e + LAGP2 + 2].append(("ln1a", l, gg))
            sched[e + LAGP2 + 3].append(("ln1b", l, gg))
            sched[e + LAGP2 + 4].append(("ln1c", l, gg))
            if l < L - 1:
                # the last layer is pooled straight from node-major hn
                sched[e + LAGP2 + 5].append(("ln2", l, gg))
    FN = dict(m12a=emit_m12a, m12b=emit_m12b, h=emit_h,
              ln1a=emit_ln1a, ln1b=emit_ln1b, ln1c=emit_ln1c, ln2=emit_ln2)
    TOT = max(sched.keys()) + 1 if sched else 0
    TOT = max(TOT, J + LAGP2 + 1)
    for i in range(TOT):
        acts = sched.pop(i, [])
        for a in acts:                      # m12 prefetches first
            if a[0].startswith("m12"):
                FN[a[0]](a[1], a[2])
        # p1/p2 before DR: frees their PSUM ring slots in PE program order
        if LAGP1 <= i < J + LAGP1:
            emit_p1(i - LAGP1)
        if LAGP2 <= i < J + LAGP2:
            emit_p2(i - LAGP2)
        for a in acts:                      # h/LN stages after p2+reduce
            if not a[0].startswith("m12"):
                FN[a[0]](a[1], a[2])
        if i < J:
            emit_dr(i)
    hid_prev = layer_st[L - 1]["hid"]

    # ---- pooling + prediction MLP: graph_emb[f] = mean_n hid[n, f]
    # pooled directly from node-major hn via ones-column matmuls
    ge_ps = mps("ge_ps")
    for gg in range(BL):
        nc.tensor.matmul(ge_ps[:, gg:gg + 1], graph_st[(L - 1, gg)]["hn"][:],
                         W("ones"), start=True, stop=True)
    ge = pLN.tile([128, BL], F32R, name="ge", tag="ge")
    nc.scalar.activation(ge[:], ge_ps[:, 0:BL], AF.Copy, scale=1.0 / N)
    o1 = mps("o1_ps")
    nc.tensor.matmul(o1[:, 0:BL], whb_sb[:, 0:H], ge[:], start=True, stop=True)
    t1 = pLN.tile([128, BL], F32R, name="t1", tag="t1")
    nc.scalar.activation(t1[:], o1[:, 0:BL], AF.Relu,
                         bias=bc_sb[:, 3 * L + 1:3 * L + 2])
    o2 = mps("o2_ps")
    nc.tensor.matmul(o2[:, 0:BL], whb_sb[:, H:2 * H], t1[:],
                     start=True, stop=True)
    out_sb = pLN.tile([OUT, BL], F32, name="out_sb", tag="out_sb")
    nc.scalar.activation(out_sb[:], o2[:, 0:BL], AF.Identity,
                         bias=bh2_sb[:])
    nc.sync.dma_start(d["d_out"].ap(), out_sb[:])
    ctx.close()


# --------------------------------------------------------------------------
# Entry point.
# --------------------------------------------------------------------------

def build(inputs):
    struct, percore = _prep(inputs)
    A = _weight_arrays(inputs)
    wmap = A.pop("_wmap")
    key = (struct["S_graph"], struct["tiles"], struct["has_empty"])
    if key not in _CACHE:
        _CACHE[key] = _build_program(struct, wmap, A["wblob"].shape[1])
    nc = _CACHE[key]

    in_maps = []
    atomb = A.pop("atomb")
    for c in range(M):
        im = dict(
            gsd=percore["gsd"][c], soh2=percore["soh2"][c],
            axoh=np.ascontiguousarray(
                np.concatenate([atomb, percore["xoh"][c]], 1)),
            maskrow=percore["maskrow"][c:c + 1],
            negrow=percore["negrow"][c:c + 1],
        )
        for k, v in A.items():
            im[k] = v
        in_maps.append(im)
    return nc, in_maps, struct


def kernel(**inputs):
    from concourse import bass_utils
    nc, in_maps, struct = build(inputs)
    res = bass_utils.run_bass_kernel_spmd(nc, in_maps, core_ids=list(range(M)))
    out = np.zeros((B, OUT), np.float32)
    for c in range(M):
        out[c * BL:(c + 1) * BL] = res.results[c]["out"].T
    return out


# revision 62
# speedup vs baseline: 1.0085x; 1.0085x over previous
"""Trainium2 Bass kernel for nn_BaselineModel_74509092651544 (CLRS-style MPNN).

Strategy
--------
Data-parallel over graphs: 32 graphs -> 8 cores x 4 graphs.  The dense
[B,N,N,H] message tensor of the reference is never materialized: only the
~62k unique (graph,src,dst) edge slots survive the masked max, so the
message MLP runs on a padded uniform-K slot layout (~8.5x less compute).

Per core, everything lives in SBUF feature-major [H=128, cols]:
  * node embeddings via fp8 DoubleRow one-hot matmuls (hi+res two-term
    split keeps ~bf16 accuracy at fp8 double rate),
  * m1[src]+m2[dst]+edge_fts@We accumulated in PSUM via 2 fp8 DoubleRow
    matmuls per 256-slot chunk: (m1,m2)@(Gsrc,Gdst) pairs the two gather
    matrices; (bw0,bw1)@(soh0,soh1) splits the 48-dim bond one-hot in two
    24-dim halves.  Gather/one-hot matrices are host-built integers.
  * the 2-layer message MLP as bf16 matmuls at up to N=512,
  * masked max over senders as one DVE segmented reduce per uniform-K tile,
  * PSUM->SBUF relu/copy traffic load-balanced across ACT/DVE/Pool engines,
  * LayerNorm via PE transposes + DVE bn_stats; ln scale/bias folded into
    the next layer's weights host-side (exact algebra),
  * graph pooling + prediction MLP on-device; output is [OUT, 4] per core.

Host work is integer indexing / relayout plus parameter-only folds
(bias sums, ln folding, bond_emb@We).  All data-dependent float math
happens on device.
"""

import sys
import numpy as np

sys.path.insert(0, "/opt/trn_rl_repo")

B, N, H, L, E, OUT = 32, 128, 128, 3, 65536, 128
M = 8                 # NeuronCores
BL = B // M           # graphs per core
NEG = -1e9
EPS = 1e-5
AV, BV = 128, 16

MT_HIRES = False    # two-term fp8 m1/m2 split (extra accuracy, ~5us slower)
_CACHE = {}


# --------------------------------------------------------------------------
# Host preprocessing: integer indexing / relayout.
# --------------------------------------------------------------------------

def _prep(inputs):
    import ml_dtypes
    FP8 = ml_dtypes.float8_e4m3fn

    x = np.asarray(inputs["x"]).astype(np.int64)            # [B*N, 9]
    ea = np.asarray(inputs["edge_attr"]).astype(np.int64)   # [E, 3]
    ei = np.asarray(inputs["edge_index"]).astype(np.int64)  # [2, E]

    g = ei[0] // N
    s = ei[0] % N
    d = ei[1] % N
    key = (g * N + s) * N + d
    uniq, inv = np.unique(key, return_inverse=True)
    US = uniq.size
    ug = uniq // (N * N)
    us = (uniq // N) % N
    ud = uniq % N

    # bond one-hot counts per unique slot  [US, 48]
    oh48 = np.zeros((US, 48), np.float32)
    for c in range(3):
        np.add.at(oh48, (inv, ea[:, c] + 16 * c), 1.0)
    assert float(oh48.max()) <= 16.0

    # unique in-degree per (graph, receiver)
    deg = np.zeros((B, N), np.int64)
    np.add.at(deg, (ug, ud), 1)

    # receiver relabeling: position p holds the p-th highest-degree receiver
    rho = np.argsort(-deg, axis=1, kind="stable")        # [B, N] pos -> orig
    rho_inv = np.argsort(rho, axis=1)                    # orig -> pos
    degS = -np.sort(-deg, axis=1)                        # [B, N] desc
    Kp = np.maximum(degS.max(axis=0), 1)                 # [N]

    # uniform-K tiles: positions p0..p0+R-1 all padded to K = Kp[p0]
    tiles = []
    p = 0
    while p < N:
        K = int(Kp[p])
        R = min(512 // K, N - p)
        tiles.append((p, R, K))
        p += R
    n_tiles = len(tiles)
    S_graph = 512 * n_tiles
    S_core = BL * S_graph
    tile_w = [R * K for (_, R, K) in tiles]

    tile_of_pos = np.zeros(N, np.int64)
    K_of_pos = np.zeros(N, np.int64)
    p0_of_pos = np.zeros(N, np.int64)
    for t, (p0, R, K) in enumerate(tiles):
        tile_of_pos[p0:p0 + R] = t
        K_of_pos[p0:p0 + R] = K
        p0_of_pos[p0:p0 + R] = p0

    # slots ordered by (g, d, s): contiguous per receiver
    order = np.lexsort((us, ud, ug))
    og, od, osl = ug[order], ud[order], order
    osrc = us[order]
    recv_id = og * N + od
    first = np.concatenate([[0], np.flatnonzero(np.diff(recv_id)) + 1])
    k_rank = np.arange(len(og)) - first[np.searchsorted(recv_id[first], recv_id)]

    pos = rho_inv[og, od]

    # padding: receivers with deg < K duplicate their first slot
    fg, fd = og[first], od[first]
    fpos = rho_inv[fg, fd]
    fdeg = deg[fg, fd]
    fK = K_of_pos[fpos]
    padc = (fK - fdeg).astype(np.int64)
    assert (padc >= 0).all()
    rep = np.repeat(np.arange(len(first)), padc)
    kpad = np.arange(len(rep)) - np.repeat(
        np.concatenate([[0], np.cumsum(padc)[:-1]]), padc
    ) + np.repeat(fdeg, padc)
    pg = fg[rep]
    pos_p = fpos[rep]
    src_p = osrc[first][rep]
    slot_p = osl[first][rep]

    a_g = np.concatenate([og, pg])
    a_pos = np.concatenate([pos, pos_p])
    a_k = np.concatenate([k_rank, kpad])
    a_slot = np.concatenate([osl, slot_p])
    a_srcnew = np.concatenate([rho_inv[og, osrc], rho_inv[pg, src_p]])

    # slot -> chunked gsd/soh2 columns
    a_t = tile_of_pos[a_pos]
    a_w = np.asarray(tile_w)[a_t]
    a_x = (a_pos - p0_of_pos[a_pos]) * K_of_pos[a_pos] + a_k
    wc0 = np.minimum(a_w, 256)
    a_c = (a_x >= wc0).astype(np.int64)          # chunk index (0 or 1)
    a_xi = a_x - a_c * 256
    a_wcc = np.where(a_c == 0, wc0, a_w - 256)
    gl = a_g % BL
    core = a_g // BL
    gbase = 2 * (gl * S_graph + a_t * 512) + a_c * 512
    col_src = gbase + a_xi
    col_dst = gbase + a_wcc + a_xi

    flat_src = core * (2 * S_core) + col_src
    flat_dst = core * (2 * S_core) + col_dst

    Gsd = np.zeros((M * 2 * S_core, 128), np.float32)
    Gsd[flat_src, a_srcnew] = 1.0
    a_dstpos = a_pos
    Gsd[flat_dst, a_dstpos] = 1.0
    Gsd = np.ascontiguousarray(
        Gsd.reshape(M, 2 * S_core, 128).transpose(0, 2, 1)).astype(FP8)

    # soh2: rows 0:24 hold bond-dims 0:24 at src-half columns and bond-dims
    # 24:48 at dst-half columns; values are counts/16 (exact in fp8).
    SOH2 = np.zeros((M * 2 * S_core, 24), np.float32)
    SOH2[flat_src] = oh48[a_slot, 0:24] * (1.0 / 16.0)
    SOH2[flat_dst] = oh48[a_slot, 24:48] * (1.0 / 16.0)
    SOH2 = np.ascontiguousarray(
        SOH2.reshape(M, 2 * S_core, 24).transpose(0, 2, 1)).astype(FP8)

    # atom one-hot per core: [9, 128, BL*N] in relabeled node order
    gg = np.repeat(np.arange(B), N)
    pp = np.tile(np.arange(N), B)
    orig = gg * N + rho[gg, pp]                    # [B*N] column -> orig node
    XOH = np.zeros((M, 9, AV, BL * N), np.float32)
    mcol = np.tile(np.arange(BL * N), M)
    mcore = np.repeat(np.arange(M), BL * N)
    for c in range(9):
        XOH[mcore, c, x[orig, c], mcol] = 1.0
    # [M, AV, 9*BL*N] c-major columns: one resident DMA instead of a
    # double-buffered stream (the 2-deep ring serialized the fill phase)
    XOH = np.ascontiguousarray(XOH.transpose(0, 2, 1, 3).reshape(
        M, AV, 9 * BL * N)).astype(FP8)

    # empty receivers (deg==0) -> NEG mask path
    empt = (deg == 0)
    has_empty = bool(empt.any())
    maskrow = np.ones((M, BL * N), np.float32)
    negrow = np.zeros((M, BL * N), np.float32)
    if has_empty:
        eg, en = np.nonzero(empt)
        epos = rho_inv[eg, en]
        maskrow[eg // BL, (eg % BL) * N + epos] = 0.0
        negrow[eg // BL, (eg % BL) * N + epos] = NEG

    struct = dict(
        S_graph=S_graph, S_core=S_core, n_tiles=n_tiles,
        tiles=tuple(tiles), tile_w=tuple(tile_w), has_empty=has_empty,
    )
    percore = dict(gsd=Gsd, soh2=SOH2, xoh=XOH,
                   maskrow=maskrow, negrow=negrow)
    return struct, percore


def _weight_arrays(inputs):
    import ml_dtypes
    f32 = np.float32
    BF16 = ml_dtypes.bfloat16
    FP8 = ml_dtypes.float8_e4m3fn
    A = {}

    Wm1 = np.asarray(inputs["Wm1"], f32)
    Wm2 = np.asarray(inputs["Wm2"], f32)
    We = np.asarray(inputs["We"], f32)
    Wo1 = np.asarray(inputs["Wo1"], f32)
    Wo2 = np.asarray(inputs["Wo2"], f32)
    Wp1 = np.asarray(inputs["Wp1"], f32)
    Wp2 = np.asarray(inputs["Wp2"], f32)
    ln_s = np.asarray(inputs["ln_s"], f32)
    ln_b = np.asarray(inputs["ln_b"], f32)
    bond = np.asarray(inputs["bond_emb"], f32).reshape(48, H)
    atom = np.asarray(inputs["atom_emb"], f32)

    # ln fold: hidden entering layer l is hid_used = hnorm*ln_s[l-1]+ln_b[l-1]
    # (exact algebra; layer 0 uses raw zeros -> identity fold)
    lnS = np.concatenate([np.ones((1, H), f32), ln_s[:L - 1]], 0)   # [L, H]
    lnB = np.concatenate([np.zeros((1, H), f32), ln_b[:L - 1]], 0)

    # weight blob (bf16, one DMA): per-layer stationary/moving tiles
    cols = []
    wmap = {}

    def add(name, arr):
        wmap[name] = (sum(c.shape[1] for c in cols), arr.shape[1])
        cols.append(np.asarray(arr, f32))

    add("idn", np.eye(128, dtype=f32))
    add("ones", np.ones((128, 1), f32))
    for l in range(L):
        m1t, m1b = Wm1[l, 0:128], Wm1[l, 128:256] * lnS[l][:, None]
        m2t, m2b = Wm2[l, 0:128], Wm2[l, 128:256] * lnS[l][:, None]
        add(f"m12_{l}_0", np.concatenate([m1t, m2t], 1))
        add(f"m12_{l}_1", np.concatenate([m1b, m2b], 1))
        add(f"Wp1_{l}", Wp1[l])
        add(f"Wp2_{l}", Wp2[l])
        add(f"Wo1_{l}_0", Wo1[l, 0:128])
        add(f"Wo1_{l}_1", Wo1[l, 128:256] * lnS[l][:, None])
        add(f"Wo2_{l}", Wo2[l])
    A["wblob"] = np.ascontiguousarray(np.concatenate(cols, 1)).astype(BF16)
    A["_wmap"] = wmap

    # head weights stay f32r: their error would hit the output directly
    # (plus one f32r ones-column for the layernorm sum-of-squares matmul)
    Wh1 = np.asarray(inputs["Wh1"], f32)
    A["whb"] = np.ascontiguousarray(np.concatenate(
        [ln_s[L - 1][:, None] * Wh1, np.asarray(inputs["Wh2"], f32),
         np.ones((H, 1), f32)], 1))

    # atom embeddings: fp8 hi/res pairs of 16*atom_emb [AV, 9*256]
    # (x16 keeps the residual term out of fp8 subnormals; nf copy scales back)
    at = atom.transpose(1, 0, 2) * 16.0                # [AV, 9, H]
    hi = at.astype(FP8)
    res = (at - hi.astype(f32)).astype(FP8)
    pair = np.concatenate([hi[:, :, None, :], res[:, :, None, :]], 2)
    A["atomb"] = np.ascontiguousarray(pair.reshape(AV, 9 * 2 * H))

    # bond tables: bw2[l] = [We[l]^T rows for bond dims 0:24 | 24:48] * 16
    bw = np.einsum("bh,lhf->lbf", bond, We)            # [L, 48, H]
    bw2 = np.concatenate([bw[:, 0:24], bw[:, 24:48]], 2) * 16.0  # [L,24,2H]
    A["bw2"] = np.ascontiguousarray(
        bw2.transpose(1, 0, 2).reshape(24, L * 2 * H)).astype(FP8)

    # bias columns [128, 3L+3]: bias_pre(L), bp1(L), bias_h(L), eps, bh1, bh2h
    bm1 = np.asarray(inputs["bm1"], f32)
    bm2 = np.asarray(inputs["bm2"], f32)
    be = np.asarray(inputs["be"], f32)
    bg = np.asarray(inputs["bg"], f32)
    bp1 = np.asarray(inputs["bp1"], f32)
    bp2 = np.asarray(inputs["bp2"], f32)
    bo1 = np.asarray(inputs["bo1"], f32)
    bo2 = np.asarray(inputs["bo2"], f32)
    bias_pre = (bm1 + bm2 + be + bg
                + np.einsum("lh,lhf->lf", lnB,
                            Wm1[:, 128:256] + Wm2[:, 128:256]))     # [L, H]
    bias_h = (bo1 + bo2 + np.einsum("lh,lhf->lf", bp2, Wo2)
              + np.einsum("lh,lhf->lf", lnB, Wo1[:, 128:256]))      # [L, H]
    bh1_eff = np.asarray(inputs["bh1"], f32) + ln_b[L - 1] @ Wh1

    bc = np.zeros((H, 3 * L + 3), f32)
    bc[:, 0:L] = bias_pre.T
    bc[:, L:2 * L] = bp1.T
    bc[:, 2 * L:3 * L] = bias_h.T
    bc[:, 3 * L] = EPS
    bc[:, 3 * L + 1] = bh1_eff
    bc[:, 3 * L + 2] = np.asarray(inputs["bh2"], f32)[:H]
    A["bias_cols"] = bc
    A["bh2_full"] = np.ascontiguousarray(
        np.asarray(inputs["bh2"], f32).reshape(OUT, 1))
    return A


# --------------------------------------------------------------------------
# Bass program.
# --------------------------------------------------------------------------

def _build_program(struct, wmap, wcols):
    import concourse.bacc as bacc
    import concourse.mybir as mybir
    import concourse.tile as tile

    F32 = mybir.dt.float32
    BF16 = mybir.dt.bfloat16
    FP8 = mybir.dt.float8e4

    S_core = struct["S_core"]

    nc = bacc.Bacc("TRN2", target_bir_lowering=False, debug=False)

    d = {}
    d["d_gsd"] = nc.dram_tensor("gsd", [128, 2 * S_core], FP8,
                                kind="ExternalInput")
    d["d_soh2"] = nc.dram_tensor("soh2", [24, 2 * S_core], FP8,
                                 kind="ExternalInput")
    d["d_axoh"] = nc.dram_tensor("axoh", [AV, 9 * 2 * H + 9 * BL * N], FP8,
                                 kind="ExternalInput")
    d["d_bw2"] = nc.dram_tensor("bw2", [24, L * 2 * H], FP8,
                                kind="ExternalInput")
    d["d_wblob"] = nc.dram_tensor("wblob", [128, wcols], BF16,
                                  kind="ExternalInput")
    d["d_whb"] = nc.dram_tensor("whb", [128, 2 * H + 1], mybir.dt.float32r,
                                kind="ExternalInput")
    d["d_bc"] = nc.dram_tensor("bias_cols", [H, 3 * L + 3], F32,
                               kind="ExternalInput")
    d["d_bh2"] = nc.dram_tensor("bh2_full", [OUT, 1], F32,
                                kind="ExternalInput")
    d["d_mask"] = nc.dram_tensor("maskrow", [1, BL * N], F32,
                                 kind="ExternalInput")
    d["d_neg"] = nc.dram_tensor("negrow", [1, BL * N], F32,
                                kind="ExternalInput")
    d["d_out"] = nc.dram_tensor("out", [OUT, BL], F32, kind="ExternalOutput")

    with tile.TileContext(nc) as tc:
        _emit(tc, nc, d, struct, wmap, mybir)
    nc.compile()
    return nc


def _emit(tc, nc, d, struct, wmap, mybir):
    import contextlib
    ctx = contextlib.ExitStack()
    F32 = mybir.dt.float32
    BF16 = mybir.dt.bfloat16
    FP8 = mybir.dt.float8e4
    AF = mybir.ActivationFunctionType
    ALU = mybir.AluOpType
    AX = mybir.AxisListType
    DR = mybir.MatmulPerfMode.DoubleRow

    S_graph = struct["S_graph"]
    S_core = struct["S_core"]
    n_tiles = struct["n_tiles"]
    tiles = struct["tiles"]
    tile_w = struct["tile_w"]
    has_empty = struct["has_empty"]

    pG = ctx.enter_context(tc.tile_pool(name="pG", bufs=1))
    pW = ctx.enter_context(tc.tile_pool(name="pW", bufs=1))
    pAct = ctx.enter_context(tc.tile_pool(name="pAct", bufs=6))
    pNM = ctx.enter_context(tc.tile_pool(name="pNM", bufs=1))
    pMB = ctx.enter_context(tc.tile_pool(name="pMB", bufs=2))
    pLN = ctx.enter_context(tc.tile_pool(name="pLN", bufs=2))
    pIn = ctx.enter_context(tc.tile_pool(name="pIn", bufs=2))
    ps_pre = ctx.enter_context(tc.tile_pool(name="ps_pre", bufs=2, space="PSUM"))
    ps_p1 = ctx.enter_context(tc.tile_pool(name="ps_p1", bufs=2, space="PSUM"))
    ps_p2 = ctx.enter_context(tc.tile_pool(name="ps_p2", bufs=2, space="PSUM"))
    ps_misc = ctx.enter_context(tc.tile_pool(name="ps_misc", bufs=2, space="PSUM"))

    def mps(name, dt=F32):
        return ps_misc.tile([128, 512], dt, name=name, tag="mps")

    # ---- engine load balancer for PSUM->SBUF relu/copy traffic (ns units)
    eng_load = {"act": 0.0, "dve": 0.0, "pool": 0.0}
    ECOST = {"act": lambda w: w * 0.833 + 170.0,
             "dve": lambda w: w * 1.0417 + 125.0,
             "pool": lambda w: w * 1.39 + 131.0}

    def assign(w, only=None, engines=("act", "dve")):
        # GPSIMD/Pool cannot access PSUM on TRN2, so PSUM-draining work is
        # restricted to ACT/DVE; SBUF-only ops may pass engines with "pool"
        if only is not None:
            e = only
        else:
            e = min(engines, key=lambda k: eng_load[k] + ECOST[k](w))
        eng_load[e] += ECOST[e](w)
        return e

    def emit_relu(out, in_, w, bias, only=None):
        e = assign(w, only)
        if e == "act":
            nc.scalar.activation(out, in_, AF.Relu, bias=bias)
        elif e == "dve":
            nc.vector.tensor_scalar(out, in_, bias, 0.0,
                                    op0=ALU.add, op1=ALU.max)
        else:
            nc.gpsimd.tensor_scalar(out, in_, bias, 0.0,
                                    op0=ALU.add, op1=ALU.max)

    def emit_copy(out, in_, w, only=None):
        e = assign(w, only)
        if e == "act":
            nc.scalar.activation(out, in_, AF.Copy)
        elif e == "dve":
            nc.vector.tensor_copy(out, in_)
        else:
            nc.gpsimd.tensor_copy(out, in_)

    def charge_ns(e, ns):
        eng_load[e] += ns

    # ---- resident loads (small/early-needed tensors first)
    bc_sb = pW.tile([H, 3 * L + 3], F32, name="bc_sb")
    nc.sync.dma_start(bc_sb[:], d["d_bc"].ap())
    wcols = sum(w for (_, w) in wmap.values())
    wblob_sb = pW.tile([128, wcols], BF16, name="wblob_sb")
    nc.sync.dma_start(wblob_sb[:], d["d_wblob"].ap())
    atomb_sb = pW.tile([AV, 9 * 2 * H], FP8, name="atomb_sb")
    nc.sync.dma_start(atomb_sb[:], d["d_atomb"].ap())
    bw2_sb = pW.tile([24, L * 2 * H], FP8, name="bw2_sb")
    nc.sync.dma_start(bw2_sb[:], d["d_bw2"].ap())
    bh2_sb = pW.tile([OUT, 1], F32, name="bh2_sb")
    nc.sync.dma_start(bh2_sb[:], d["d_bh2"].ap())
    F32R = mybir.dt.float32r
    whb_sb = pW.tile([128, 2 * H + 1], F32R, name="whb_sb")
    nc.sync.dma_start(whb_sb[:], d["d_whb"].ap())

    def W(name):
        off, w = wmap[name]
        return wblob_sb[:, off:off + w]

    idn_sb = W("idn")

    if has_empty:
        mrow_sb = pW.tile([1, BL * N], F32, name="mrow_sb")
        nc.sync.dma_start(mrow_sb[:], d["d_mask"].ap())
        nrow_sb = pW.tile([1, BL * N], F32, name="nrow_sb")
        nc.sync.dma_start(nrow_sb[:], d["d_neg"].ap())
        mask_bc = pW.tile([128, BL * N], F32, name="mask_bc")
        nc.gpsimd.partition_broadcast(mask_bc[:], mrow_sb[:])
        neg_bc = pW.tile([128, BL * N], F32, name="neg_bc")
        nc.gpsimd.partition_broadcast(neg_bc[:], nrow_sb[:])

    gsd_sb = pG.tile([128, 2 * S_core], FP8, name="gsd_sb")
    soh2_sb = pG.tile([24, 2 * S_core], FP8, name="soh2_sb")

    # ---- node features via fp8 DoubleRow (hi,res) pairs; hidden0 = 0
    xoh_sb = axoh_sb[:, 9 * 2 * H:]
    nf_ps = mps("nf_ps")
    nf = pNM.tile([128, BL * N], BF16, name="nf")
    for half in range(2):
        # half-major order with a split copy: the first m1/m2 block only
        # needs graph 0's node columns, so layer 0 starts ~2us earlier
        for c in range(9):
            st = atomb_sb[:, c * 2 * H:(c + 1) * 2 * H].rearrange(
                "p (two m) -> p two m", two=2)
            co = c * BL * N + half * 256
            mv = xoh_sb[:, co:co + 256].rearrange(
                "p (one n) -> p one n", one=1).broadcast_to([AV, 2, 256])
            nc.tensor.matmul(nf_ps[:, half * 256:(half + 1) * 256], st, mv,
                             start=(c == 0), stop=(c == 8), perf_mode=DR)
        nc.scalar.activation(nf[:, half * 256:(half + 1) * 256],
                             nf_ps[:, half * 256:(half + 1) * 256],
                             AF.Copy, scale=1.0 / 16.0)
    hid0 = pNM.tile([128, BL * N], BF16, name="hid0", tag="hid0")
    nc.vector.memset(hid0[:], 0.0)
    # prime the ACT Sqrt table while the engine is idle (a mid-run table
    # load costs 1283ns on the saturated ACT engine)
    warm = pLN.tile([128, 1], F32, name="warm", tag="warm")
    nc.scalar.activation(warm[:], bc_sb[:, 3 * L:3 * L + 1], AF.Sqrt)

    for gg in range(BL):
        sl = slice(2 * gg * S_graph, 2 * (gg + 1) * S_graph)
        nc.sync.dma_start(gsd_sb[:, sl], d["d_gsd"].ap()[:, sl])
        nc.sync.dma_start(soh2_sb[:, sl], d["d_soh2"].ap()[:, sl])

    hid_prev = hid0

    # ----------------------------------------------------------------
    # Globally software-pipelined job loop: jobs = (layer, graph, tile).
    # Stage lags keep the in-order PE stream busy: at step i the PE runs
    # DR(i), p1(i-1), p2(i-2) while ACT/DVE/Pool run the relus of earlier
    # jobs, so no engine ever waits head-of-line on a cross-engine hop.
    # ----------------------------------------------------------------
    jobs = [(l, gg, t) for l in range(L) for gg in range(BL)
            for t in range(n_tiles)]
    J = len(jobs)
    st8 = {}          # per-job state: pre/p1/p2/msgs tiles
    layer_st = {}     # per-layer tiles (msgs_max, h_fm, hid)
    graph_st = {}     # per-(layer,graph) tiles (mt, hn, ...)

    def layer_state(l):
        if l in layer_st:
            return layer_st[l]
        s = dict(
            msgs_max=pLN.tile([128, BL * N], BF16, name=f"msgs_max{l}",
                              tag=f"msgs_max{l % 2}", bufs=1),
            h_fm=pLN.tile([128, BL * N], BF16, name=f"h_fm{l}",
                          tag=f"h_fm{l % 2}", bufs=1),
            hid=(pNM.tile([128, BL * N], BF16, name=f"hid{l + 1}",
                          tag=f"hid{(l + 1) % 2}") if l < L - 1 or True
                 else None),
            bw2_l=bw2_sb[:, l * 2 * H:(l + 1) * 2 * H].rearrange(
                "p (two m) -> p two m", two=2),
        )
        layer_st[l] = s
        return s

    def emit_m12a(l, gg):
        ls = layer_state(l)
        hid_in = hid0 if l == 0 else layer_st[l - 1]["hid"]
        gsl = slice(gg * N, (gg + 1) * N)
        ps_m = mps("ps_m")
        nc.tensor.matmul(ps_m[:, 0:2 * H], nf[:, gsl],
                         W(f"m12_{l}_0"), start=True, stop=(l == 0))
        if l > 0:
            nc.tensor.matmul(ps_m[:, 0:2 * H], hid_in[:, gsl],
                             W(f"m12_{l}_1"), start=False, stop=True)
        # optional hi/res two-term fp8 split of (m1|m2) (~bf16 accuracy)
        mt = pMB.tile([128, 4 * H], FP8, name=f"mt{gg}", tag=f"mt{gg}",
                      bufs=2)
        emit_copy(mt[:, 0:2 * H], ps_m[:, 0:2 * H], 2 * H)
        graph_st[(l, gg)] = dict(
            ps_m=ps_m, mt=mt,
            mt_hi=mt[:, 0:2 * H].rearrange("p (two m) -> p two m", two=2),
            mt_res=mt[:, 2 * H:4 * H].rearrange("p (two m) -> p two m",
                                                two=2))

    def emit_m12b(l, gg):
        if not MT_HIRES:
            return
        gs = graph_st[(l, gg)]
        mt, ps_m = gs["mt"], gs["ps_m"]
        nc.vector.tensor_tensor(mt[:, 2 * H:4 * H], ps_m[:, 0:2 * H],
                                mt[:, 0:2 * H], op=ALU.subtract)
        charge_ns("dve", 2 * H * 1.0417 + 125.0)

    def emit_dr(i):
        l, gg, t = jobs[i]
        gs = graph_st[(l, gg)]
        ls = layer_state(l)
        w = tile_w[t]
        base = 2 * (gg * S_graph + t * 512)
        pre = ps_pre.tile([128, 512], F32, name="pre")
        nchunks = 1 if w <= 256 else 2
        chunks = []
        for c in range(nchunks):
            wc = min(w, 256) if c == 0 else w - 256
            chunks.append((slice(c * 256, c * 256 + wc), base + c * 512, wc))
        # grouped by stationary operand to minimize LdWeights swaps; one
        # accumulation group per PSUM bank (start arms the whole 2KB
        # region, chunk 1's first write replaces, later terms accumulate)
        stns = (gs["mt_hi"], gs["mt_res"]) if MT_HIRES else (gs["mt_hi"],)
        for si, stn in enumerate(stns):
            for ci, (cr, go, wc) in enumerate(chunks):
                mv_g = gsd_sb[:, go:go + 2 * wc].rearrange(
                    "p (two n) -> p two n", two=2)
                nc.tensor.matmul(pre[:, cr], stn, mv_g,
                                 start=(si == 0 and ci == 0),
                                 stop=False, perf_mode=DR)
        for ci, (cr, go, wc) in enumerate(chunks):
            mv_s = soh2_sb[:, go:go + 2 * wc].rearrange(
                "p (two n) -> p two n", two=2)
            nc.tensor.matmul(pre[:, cr], ls["bw2_l"], mv_s,
                             start=False, stop=(ci == len(chunks) - 1),
                             perf_mode=DR)
        msgs1 = pAct.tile([128, 512], BF16, name="msgs1", tag="msgs1")
        emit_relu(msgs1[:, 0:w], pre[:, 0:w], w, bc_sb[:, l:l + 1])
        st8[i] = dict(msgs1=msgs1)

    def emit_p1(i):
        l, gg, t = jobs[i]
        w = tile_w[t]
        p1 = ps_p1.tile([128, 512], F32, name="p1")
        nc.tensor.matmul(p1[:, 0:w], W(f"Wp1_{l}"), st8[i]["msgs1"][:, 0:w],
                         start=True, stop=True)
        msgs2 = pAct.tile([128, 512], BF16, name="msgs2", tag="msgs2")
        emit_relu(msgs2[:, 0:w], p1[:, 0:w], w, bc_sb[:, L + l:L + l + 1])
        st8[i]["msgs2"] = msgs2

    def emit_p2(i):
        l, gg, t = jobs[i]
        ls = layer_st[l]
        w = tile_w[t]
        p2 = ps_p2.tile([128, 512], F32, name="p2")
        nc.tensor.matmul(p2[:, 0:w], W(f"Wp2_{l}"), st8[i]["msgs2"][:, 0:w],
                         start=True, stop=True)
        (p0, R, K) = tiles[t]
        nc.vector.tensor_reduce(
            ls["msgs_max"][:, gg * N + p0: gg * N + p0 + R],
            p2[:, 0:w].rearrange("p (r k) -> p r k", r=R),
            axis=AX.X, op=ALU.max)
        charge_ns("dve", w * 1.0417 + 125.0)
        del st8[i]

    def emit_h(l, gg):
        ls = layer_st[l]
        hid_in = hid0 if l == 0 else layer_st[l - 1]["hid"]
        gsl = slice(gg * N, (gg + 1) * N)
        msl = ls["msgs_max"][:, gsl]
        if has_empty:
            mm1 = pLN.tile([128, N], F32, name="mm1", tag="mm1")
            nc.vector.tensor_tensor(mm1[:], msl, mask_bc[:, gsl],
                                    op=ALU.mult)
            nc.vector.tensor_tensor(mm1[:], mm1[:], neg_bc[:, gsl],
                                    op=ALU.add)
            mm2 = pLN.tile([128, N], BF16, name="mm2", tag="mm2")
            nc.vector.tensor_copy(mm2[:], mm1[:])
            msl = mm2[:]
        h_ps = mps("h_ps")
        nc.tensor.matmul(h_ps[:, 0:N], W(f"Wo1_{l}_0"), nf[:, gsl],
                         start=True, stop=False)
        nc.tensor.matmul(h_ps[:, 0:N], W(f"Wo1_{l}_1"), hid_in[:, gsl],
                         start=False, stop=False)
        nc.tensor.matmul(h_ps[:, 0:N], W(f"Wo2_{l}"), msl,
                         start=False, stop=True)
        emit_relu(ls["h_fm"][:, gsl], h_ps[:, 0:N], N,
                  bc_sb[:, 2 * L + l:2 * L + l + 1])

    def emit_ln1a(l, gg):
        # transpose to node-major; bn_stats reads the PSUM tile directly
        ls = layer_st[l]
        gsl = slice(gg * N, (gg + 1) * N)
        tp = mps("tp_ps", BF16)
        nc.tensor.transpose(tp[:, 0:128], ls["h_fm"][:, gsl], idn_sb)
        bn6 = pLN.tile([128, 6], F32, name="bn6", tag=f"bn6_{gg}", bufs=1)
        nc.vector.bn_stats(bn6[:], tp[:, 0:128])
        charge_ns("dve", 260.0)
        if l == L - 1:
            # drain phase: free the PSUM ring slot fast so the remaining
            # ladders don't serialize behind each other
            hT = pLN.tile([128, 128], BF16, name="hT", tag=f"hT_{gg}",
                          bufs=1)
            emit_copy(hT[:], tp[:, 0:128], 128)
            graph_st[(l, gg)].update(tp=None, hT=hT, bn6=bn6)
        else:
            graph_st[(l, gg)].update(tp=tp, hT=None, bn6=bn6)

    def emit_ln1b(l, gg):
        gs = graph_st[(l, gg)]
        bn2 = pLN.tile([128, 2], F32, name="bn2", tag=f"bn2_{gg}", bufs=1)
        nc.vector.bn_aggr(bn2[:], gs["bn6"][:])
        std = pLN.tile([128, 1], F32, name="std", tag=f"std_{gg}", bufs=1)
        nc.scalar.activation(std[:], bn2[:, 1:2], AF.Sqrt,
                             bias=bc_sb[:, 3 * L:3 * L + 1])
        gs.update(bn2=bn2, std=std)

    def emit_ln1c(l, gg):
        # normalize on DVE straight from the bf16 PSUM transpose (2-byte
        # packed operands hit the 2x DVE mode); rstd via DVE reciprocal
        gs = graph_st[(l, gg)]
        rstd = pLN.tile([128, 1], F32, name="rstd", tag=f"rstd_{gg}",
                        bufs=1)
        nc.vector.reciprocal(rstd[:], gs["std"][:])
        hn = pLN.tile([128, 128], BF16, name="hn", tag=f"hn_{gg}", bufs=1)
        src_ap = gs["hT"][:] if gs["tp"] is None else gs["tp"][:, 0:128]
        nc.vector.tensor_scalar(hn[:], src_ap, gs["bn2"][:, 0:1],
                                rstd[:], op0=ALU.subtract, op1=ALU.mult)
        charge_ns("dve", 192.0)
        gs["hn"] = hn

    def emit_ln2(l, gg):
        # transpose back to feature-major hid
        ls = layer_st[l]
        gsl = slice(gg * N, (gg + 1) * N)
        tp2 = mps("tp2_ps", BF16)
        nc.tensor.transpose(tp2[:, 0:128], graph_st[(l, gg)]["hn"][:],
                            idn_sb)
        emit_copy(ls["hid"][:, gsl], tp2[:, 0:128], N)

    LAST = n_tiles - 1
    LAGP1, LAGP2 = 2, 3
    import collections as _c
    sched = _c.defaultdict(list)
    for j, (l, gg, t) in enumerate(jobs):
        if t == 0:
            sched[max(j - 4, 0)].append(("m12a", l, gg))
            sched[max(j - 3, 0)].append(("m12b", l, gg))
        if t == LAST:
            e = j
            sched[e + LAGP2].append(("h", l, gg))
            sched[e + LAGP2 + 2].append(("ln1a", l, gg))
            sched[e + LAGP2 + 3].append(("ln1b", l, gg))
            sched[e + LAGP2 + 4].append(("ln1c", l, gg))
            if l < L - 1:
                # the last layer is pooled straight from node-major hn
                sched[e + LAGP2 + 5].append(("ln2", l, gg))
    FN = dict(m12a=emit_m12a, m12b=emit_m12b, h=emit_h,
              ln1a=emit_ln1a, ln1b=emit_ln1b, ln1c=emit_ln1c, ln2=emit_ln2)
    TOT = max(sched.keys()) + 1 if sched else 0
    TOT = max(TOT, J + LAGP2 + 1)
    for i in range(TOT):
        acts = sched.pop(i, [])
        for a in acts:                      # m12 prefetches first
            if a[0].startswith("m12"):
                FN[a[0]](a[1], a[2])
        # p1/p2 before DR: frees their PSUM ring slots in PE program order
        if LAGP1 <= i < J + LAGP1:
            emit_p1(i - LAGP1)
        if LAGP2 <= i < J + LAGP2:
            emit_p2(i - LAGP2)
        for a in acts:                      # h/LN stages after p2+reduce
            if not a[0].startswith("m12"):
                FN[a[0]](a[1], a[2])
        if i < J:
            emit_dr(i)
    hid_prev = layer_st[L - 1]["hid"]

    # ---- pooling + prediction MLP: graph_emb[f] = mean_n hid[n, f]
    # pooled directly from node-major hn via ones-column matmuls
    ge_ps = mps("ge_ps")
    for gg in range(BL):
        nc.tensor.matmul(ge_ps[:, gg:gg + 1], graph_st[(L - 1, gg)]["hn"][:],
                         W("ones"), start=True, stop=True)
    ge = pLN.tile([128, BL], F32R, name="ge", tag="ge")
    nc.scalar.activation(ge[:], ge_ps[:, 0:BL], AF.Copy, scale=1.0 / N)
    o1 = mps("o1_ps")
    nc.tensor.matmul(o1[:, 0:BL], whb_sb[:, 0:H], ge[:], start=True, stop=True)
    t1 = pLN.tile([128, BL], F32R, name="t1", tag="t1")
    nc.scalar.activation(t1[:], o1[:, 0:BL], AF.Relu,
                         bias=bc_sb[:, 3 * L + 1:3 * L + 2])
    o2 = mps("o2_ps")
    nc.tensor.matmul(o2[:, 0:BL], whb_sb[:, H:2 * H], t1[:],
                     start=True, stop=True)
    out_sb = pLN.tile([OUT, BL], F32, name="out_sb", tag="out_sb")
    nc.scalar.activation(out_sb[:], o2[:, 0:BL], AF.Identity,
                         bias=bh2_sb[:])
    nc.sync.dma_start(d["d_out"].ap(), out_sb[:])
    ctx.close()


# --------------------------------------------------------------------------
# Entry point.
# --------------------------------------------------------------------------

def build(inputs):
    struct, percore = _prep(inputs)
    A = _weight_arrays(inputs)
    wmap = A.pop("_wmap")
    key = (struct["S_graph"], struct["tiles"], struct["has_empty"])
    if key not in _CACHE:
        _CACHE[key] = _build_program(struct, wmap, A["wblob"].shape[1])
    nc = _CACHE[key]

    in_maps = []
    atomb = A.pop("atomb")
    for c in range(M):
        im = dict(
            gsd=percore["gsd"][c], soh2=percore["soh2"][c],
            axoh=np.ascontiguousarray(
                np.concatenate([atomb, percore["xoh"][c]], 1)),
            maskrow=percore["maskrow"][c:c + 1],
            negrow=percore["negrow"][c:c + 1],
        )
        for k, v in A.items():
            im[k] = v
        in_maps.append(im)
    return nc, in_maps, struct


def kernel(**inputs):
    from concourse import bass_utils
    nc, in_maps, struct = build(inputs)
    res = bass_utils.run_bass_kernel_spmd(nc, in_maps, core_ids=list(range(M)))
    out = np.zeros((B, OUT), np.float32)
    for c in range(M):
        out[c * BL:(c + 1) * BL] = res.results[c]["out"].T
    return out


# revision 63
# speedup vs baseline: 1.0118x; 1.0034x over previous
"""Trainium2 Bass kernel for nn_BaselineModel_74509092651544 (CLRS-style MPNN).

Strategy
--------
Data-parallel over graphs: 32 graphs -> 8 cores x 4 graphs.  The dense
[B,N,N,H] message tensor of the reference is never materialized: only the
~62k unique (graph,src,dst) edge slots survive the masked max, so the
message MLP runs on a padded uniform-K slot layout (~8.5x less compute).

Per core, everything lives in SBUF feature-major [H=128, cols]:
  * node embeddings via fp8 DoubleRow one-hot matmuls (hi+res two-term
    split keeps ~bf16 accuracy at fp8 double rate),
  * m1[src]+m2[dst]+edge_fts@We accumulated in PSUM via 2 fp8 DoubleRow
    matmuls per 256-slot chunk: (m1,m2)@(Gsrc,Gdst) pairs the two gather
    matrices; (bw0,bw1)@(soh0,soh1) splits the 48-dim bond one-hot in two
    24-dim halves.  Gather/one-hot matrices are host-built integers.
  * the 2-layer message MLP as bf16 matmuls at up to N=512,
  * masked max over senders as one DVE segmented reduce per uniform-K tile,
  * PSUM->SBUF relu/copy traffic load-balanced across ACT/DVE/Pool engines,
  * LayerNorm via PE transposes + DVE bn_stats; ln scale/bias folded into
    the next layer's weights host-side (exact algebra),
  * graph pooling + prediction MLP on-device; output is [OUT, 4] per core.

Host work is integer indexing / relayout plus parameter-only folds
(bias sums, ln folding, bond_emb@We).  All data-dependent float math
happens on device.
"""

import sys
import numpy as np

sys.path.insert(0, "/opt/trn_rl_repo")

B, N, H, L, E, OUT = 32, 128, 128, 3, 65536, 128
M = 8                 # NeuronCores
BL = B // M           # graphs per core
NEG = -1e9
EPS = 1e-5
AV, BV = 128, 16

MT_HIRES = False    # two-term fp8 m1/m2 split (extra accuracy, ~5us slower)
_CACHE = {}


# --------------------------------------------------------------------------
# Host preprocessing: integer indexing / relayout.
# --------------------------------------------------------------------------

def _prep(inputs):
    import ml_dtypes
    FP8 = ml_dtypes.float8_e4m3fn

    x = np.asarray(inputs["x"]).astype(np.int64)            # [B*N, 9]
    ea = np.asarray(inputs["edge_attr"]).astype(np.int64)   # [E, 3]
    ei = np.asarray(inputs["edge_index"]).astype(np.int64)  # [2, E]

    g = ei[0] // N
    s = ei[0] % N
    d = ei[1] % N
    key = (g * N + s) * N + d
    uniq, inv = np.unique(key, return_inverse=True)
    US = uniq.size
    ug = uniq // (N * N)
    us = (uniq // N) % N
    ud = uniq % N

    # bond one-hot counts per unique slot  [US, 48]
    oh48 = np.zeros((US, 48), np.float32)
    for c in range(3):
        np.add.at(oh48, (inv, ea[:, c] + 16 * c), 1.0)
    assert float(oh48.max()) <= 16.0

    # unique in-degree per (graph, receiver)
    deg = np.zeros((B, N), np.int64)
    np.add.at(deg, (ug, ud), 1)

    # receiver relabeling: position p holds the p-th highest-degree receiver
    rho = np.argsort(-deg, axis=1, kind="stable")        # [B, N] pos -> orig
    rho_inv = np.argsort(rho, axis=1)                    # orig -> pos
    degS = -np.sort(-deg, axis=1)                        # [B, N] desc
    Kp = np.maximum(degS.max(axis=0), 1)                 # [N]

    # uniform-K tiles: positions p0..p0+R-1 all padded to K = Kp[p0]
    tiles = []
    p = 0
    while p < N:
        K = int(Kp[p])
        R = min(512 // K, N - p)
        tiles.append((p, R, K))
        p += R
    n_tiles = len(tiles)
    S_graph = 512 * n_tiles
    S_core = BL * S_graph
    tile_w = [R * K for (_, R, K) in tiles]

    tile_of_pos = np.zeros(N, np.int64)
    K_of_pos = np.zeros(N, np.int64)
    p0_of_pos = np.zeros(N, np.int64)
    for t, (p0, R, K) in enumerate(tiles):
        tile_of_pos[p0:p0 + R] = t
        K_of_pos[p0:p0 + R] = K
        p0_of_pos[p0:p0 + R] = p0

    # slots ordered by (g, d, s): contiguous per receiver
    order = np.lexsort((us, ud, ug))
    og, od, osl = ug[order], ud[order], order
    osrc = us[order]
    recv_id = og * N + od
    first = np.concatenate([[0], np.flatnonzero(np.diff(recv_id)) + 1])
    k_rank = np.arange(len(og)) - first[np.searchsorted(recv_id[first], recv_id)]

    pos = rho_inv[og, od]

    # padding: receivers with deg < K duplicate their first slot
    fg, fd = og[first], od[first]
    fpos = rho_inv[fg, fd]
    fdeg = deg[fg, fd]
    fK = K_of_pos[fpos]
    padc = (fK - fdeg).astype(np.int64)
    assert (padc >= 0).all()
    rep = np.repeat(np.arange(len(first)), padc)
    kpad = np.arange(len(rep)) - np.repeat(
        np.concatenate([[0], np.cumsum(padc)[:-1]]), padc
    ) + np.repeat(fdeg, padc)
    pg = fg[rep]
    pos_p = fpos[rep]
    src_p = osrc[first][rep]
    slot_p = osl[first][rep]

    a_g = np.concatenate([og, pg])
    a_pos = np.concatenate([pos, pos_p])
    a_k = np.concatenate([k_rank, kpad])
    a_slot = np.concatenate([osl, slot_p])
    a_srcnew = np.concatenate([rho_inv[og, osrc], rho_inv[pg, src_p]])

    # slot -> chunked gsd/soh2 columns
    a_t = tile_of_pos[a_pos]
    a_w = np.asarray(tile_w)[a_t]
    a_x = (a_pos - p0_of_pos[a_pos]) * K_of_pos[a_pos] + a_k
    wc0 = np.minimum(a_w, 256)
    a_c = (a_x >= wc0).astype(np.int64)          # chunk index (0 or 1)
    a_xi = a_x - a_c * 256
    a_wcc = np.where(a_c == 0, wc0, a_w - 256)
    gl = a_g % BL
    core = a_g // BL
    gbase = 2 * (gl * S_graph + a_t * 512) + a_c * 512
    col_src = gbase + a_xi
    col_dst = gbase + a_wcc + a_xi

    flat_src = core * (2 * S_core) + col_src
    flat_dst = core * (2 * S_core) + col_dst

    Gsd = np.zeros((M * 2 * S_core, 128), np.float32)
    Gsd[flat_src, a_srcnew] = 1.0
    a_dstpos = a_pos
    Gsd[flat_dst, a_dstpos] = 1.0
    Gsd = np.ascontiguousarray(
        Gsd.reshape(M, 2 * S_core, 128).transpose(0, 2, 1)).astype(FP8)

    # soh2: rows 0:24 hold bond-dims 0:24 at src-half columns and bond-dims
    # 24:48 at dst-half columns; values are counts/16 (exact in fp8).
    SOH2 = np.zeros((M * 2 * S_core, 24), np.float32)
    SOH2[flat_src] = oh48[a_slot, 0:24] * (1.0 / 16.0)
    SOH2[flat_dst] = oh48[a_slot, 24:48] * (1.0 / 16.0)
    SOH2 = np.ascontiguousarray(
        SOH2.reshape(M, 2 * S_core, 24).transpose(0, 2, 1)).astype(FP8)

    # atom one-hot per core: [9, 128, BL*N] in relabeled node order
    gg = np.repeat(np.arange(B), N)
    pp = np.tile(np.arange(N), B)
    orig = gg * N + rho[gg, pp]                    # [B*N] column -> orig node
    XOH = np.zeros((M, 9, AV, BL * N), np.float32)
    mcol = np.tile(np.arange(BL * N), M)
    mcore = np.repeat(np.arange(M), BL * N)
    for c in range(9):
        XOH[mcore, c, x[orig, c], mcol] = 1.0
    # [M, AV, 9*BL*N] c-major columns: one resident DMA instead of a
    # double-buffered stream (the 2-deep ring serialized the fill phase)
    XOH = np.ascontiguousarray(XOH.transpose(0, 2, 1, 3).reshape(
        M, AV, 9 * BL * N)).astype(FP8)

    # empty receivers (deg==0) -> NEG mask path
    empt = (deg == 0)
    has_empty = bool(empt.any())
    maskrow = np.ones((M, BL * N), np.float32)
    negrow = np.zeros((M, BL * N), np.float32)
    if has_empty:
        eg, en = np.nonzero(empt)
        epos = rho_inv[eg, en]
        maskrow[eg // BL, (eg % BL) * N + epos] = 0.0
        negrow[eg // BL, (eg % BL) * N + epos] = NEG

    struct = dict(
        S_graph=S_graph, S_core=S_core, n_tiles=n_tiles,
        tiles=tuple(tiles), tile_w=tuple(tile_w), has_empty=has_empty,
    )
    percore = dict(gsd=Gsd, soh2=SOH2, xoh=XOH,
                   maskrow=maskrow, negrow=negrow)
    return struct, percore


def _weight_arrays(inputs):
    import ml_dtypes
    f32 = np.float32
    BF16 = ml_dtypes.bfloat16
    FP8 = ml_dtypes.float8_e4m3fn
    A = {}

    Wm1 = np.asarray(inputs["Wm1"], f32)
    Wm2 = np.asarray(inputs["Wm2"], f32)
    We = np.asarray(inputs["We"], f32)
    Wo1 = np.asarray(inputs["Wo1"], f32)
    Wo2 = np.asarray(inputs["Wo2"], f32)
    Wp1 = np.asarray(inputs["Wp1"], f32)
    Wp2 = np.asarray(inputs["Wp2"], f32)
    ln_s = np.asarray(inputs["ln_s"], f32)
    ln_b = np.asarray(inputs["ln_b"], f32)
    bond = np.asarray(inputs["bond_emb"], f32).reshape(48, H)
    atom = np.asarray(inputs["atom_emb"], f32)

    # ln fold: hidden entering layer l is hid_used = hnorm*ln_s[l-1]+ln_b[l-1]
    # (exact algebra; layer 0 uses raw zeros -> identity fold)
    lnS = np.concatenate([np.ones((1, H), f32), ln_s[:L - 1]], 0)   # [L, H]
    lnB = np.concatenate([np.zeros((1, H), f32), ln_b[:L - 1]], 0)

    # weight blob (bf16, one DMA): per-layer stationary/moving tiles
    cols = []
    wmap = {}

    def add(name, arr):
        wmap[name] = (sum(c.shape[1] for c in cols), arr.shape[1])
        cols.append(np.asarray(arr, f32))

    add("idn", np.eye(128, dtype=f32))
    add("ones", np.ones((128, 1), f32))
    for l in range(L):
        m1t, m1b = Wm1[l, 0:128], Wm1[l, 128:256] * lnS[l][:, None]
        m2t, m2b = Wm2[l, 0:128], Wm2[l, 128:256] * lnS[l][:, None]
        add(f"m12_{l}_0", np.concatenate([m1t, m2t], 1))
        add(f"m12_{l}_1", np.concatenate([m1b, m2b], 1))
        add(f"Wp1_{l}", Wp1[l])
        add(f"Wp2_{l}", Wp2[l])
        add(f"Wo1_{l}_0", Wo1[l, 0:128])
        add(f"Wo1_{l}_1", Wo1[l, 128:256] * lnS[l][:, None])
        add(f"Wo2_{l}", Wo2[l])
    A["wblob"] = np.ascontiguousarray(np.concatenate(cols, 1)).astype(BF16)
    A["_wmap"] = wmap

    # head weights stay f32r: their error would hit the output directly
    # (plus one f32r ones-column for the layernorm sum-of-squares matmul)
    Wh1 = np.asarray(inputs["Wh1"], f32)
    A["whb"] = np.ascontiguousarray(np.concatenate(
        [ln_s[L - 1][:, None] * Wh1, np.asarray(inputs["Wh2"], f32),
         np.ones((H, 1), f32)], 1))

    # atom embeddings: fp8 hi/res pairs of 16*atom_emb [AV, 9*256]
    # (x16 keeps the residual term out of fp8 subnormals; nf copy scales back)
    at = atom.transpose(1, 0, 2) * 16.0                # [AV, 9, H]
    hi = at.astype(FP8)
    res = (at - hi.astype(f32)).astype(FP8)
    pair = np.concatenate([hi[:, :, None, :], res[:, :, None, :]], 2)
    A["atomb"] = np.ascontiguousarray(pair.reshape(AV, 9 * 2 * H))

    # bond tables: bw2[l] = [We[l]^T rows for bond dims 0:24 | 24:48] * 16
    bw = np.einsum("bh,lhf->lbf", bond, We)            # [L, 48, H]
    bw2 = np.concatenate([bw[:, 0:24], bw[:, 24:48]], 2) * 16.0  # [L,24,2H]
    A["bw2"] = np.ascontiguousarray(
        bw2.transpose(1, 0, 2).reshape(24, L * 2 * H)).astype(FP8)

    # bias columns [128, 3L+3]: bias_pre(L), bp1(L), bias_h(L), eps, bh1, bh2h
    bm1 = np.asarray(inputs["bm1"], f32)
    bm2 = np.asarray(inputs["bm2"], f32)
    be = np.asarray(inputs["be"], f32)
    bg = np.asarray(inputs["bg"], f32)
    bp1 = np.asarray(inputs["bp1"], f32)
    bp2 = np.asarray(inputs["bp2"], f32)
    bo1 = np.asarray(inputs["bo1"], f32)
    bo2 = np.asarray(inputs["bo2"], f32)
    bias_pre = (bm1 + bm2 + be + bg
                + np.einsum("lh,lhf->lf", lnB,
                            Wm1[:, 128:256] + Wm2[:, 128:256]))     # [L, H]
    bias_h = (bo1 + bo2 + np.einsum("lh,lhf->lf", bp2, Wo2)
              + np.einsum("lh,lhf->lf", lnB, Wo1[:, 128:256]))      # [L, H]
    bh1_eff = np.asarray(inputs["bh1"], f32) + ln_b[L - 1] @ Wh1

    bc = np.zeros((H, 3 * L + 3), f32)
    bc[:, 0:L] = bias_pre.T
    bc[:, L:2 * L] = bp1.T
    bc[:, 2 * L:3 * L] = bias_h.T
    bc[:, 3 * L] = EPS
    bc[:, 3 * L + 1] = bh1_eff
    bc[:, 3 * L + 2] = np.asarray(inputs["bh2"], f32)[:H]
    A["bias_cols"] = bc
    A["bh2_full"] = np.ascontiguousarray(
        np.asarray(inputs["bh2"], f32).reshape(OUT, 1))
    return A


# --------------------------------------------------------------------------
# Bass program.
# --------------------------------------------------------------------------

def _build_program(struct, wmap, wcols):
    import concourse.bacc as bacc
    import concourse.mybir as mybir
    import concourse.tile as tile

    F32 = mybir.dt.float32
    BF16 = mybir.dt.bfloat16
    FP8 = mybir.dt.float8e4

    S_core = struct["S_core"]

    nc = bacc.Bacc("TRN2", target_bir_lowering=False, debug=False)

    d = {}
    d["d_gsd"] = nc.dram_tensor("gsd", [128, 2 * S_core], FP8,
                                kind="ExternalInput")
    d["d_soh2"] = nc.dram_tensor("soh2", [24, 2 * S_core], FP8,
                                 kind="ExternalInput")
    d["d_axoh"] = nc.dram_tensor("axoh", [AV, 9 * 2 * H + 9 * BL * N], FP8,
                                 kind="ExternalInput")
    d["d_bw2"] = nc.dram_tensor("bw2", [24, L * 2 * H], FP8,
                                kind="ExternalInput")
    d["d_wblob"] = nc.dram_tensor("wblob", [128, wcols], BF16,
                                  kind="ExternalInput")
    d["d_whb"] = nc.dram_tensor("whb", [128, 2 * H + 1], mybir.dt.float32r,
                                kind="ExternalInput")
    d["d_bc"] = nc.dram_tensor("bias_cols", [H, 3 * L + 3], F32,
                               kind="ExternalInput")
    d["d_bh2"] = nc.dram_tensor("bh2_full", [OUT, 1], F32,
                                kind="ExternalInput")
    d["d_mask"] = nc.dram_tensor("maskrow", [1, BL * N], F32,
                                 kind="ExternalInput")
    d["d_neg"] = nc.dram_tensor("negrow", [1, BL * N], F32,
                                kind="ExternalInput")
    d["d_out"] = nc.dram_tensor("out", [OUT, BL], F32, kind="ExternalOutput")

    with tile.TileContext(nc) as tc:
        _emit(tc, nc, d, struct, wmap, mybir)
    nc.compile()
    return nc


def _emit(tc, nc, d, struct, wmap, mybir):
    import contextlib
    ctx = contextlib.ExitStack()
    F32 = mybir.dt.float32
    BF16 = mybir.dt.bfloat16
    FP8 = mybir.dt.float8e4
    AF = mybir.ActivationFunctionType
    ALU = mybir.AluOpType
    AX = mybir.AxisListType
    DR = mybir.MatmulPerfMode.DoubleRow

    S_graph = struct["S_graph"]
    S_core = struct["S_core"]
    n_tiles = struct["n_tiles"]
    tiles = struct["tiles"]
    tile_w = struct["tile_w"]
    has_empty = struct["has_empty"]

    pG = ctx.enter_context(tc.tile_pool(name="pG", bufs=1))
    pW = ctx.enter_context(tc.tile_pool(name="pW", bufs=1))
    pAct = ctx.enter_context(tc.tile_pool(name="pAct", bufs=6))
    pNM = ctx.enter_context(tc.tile_pool(name="pNM", bufs=1))
    pMB = ctx.enter_context(tc.tile_pool(name="pMB", bufs=2))
    pLN = ctx.enter_context(tc.tile_pool(name="pLN", bufs=2))
    pIn = ctx.enter_context(tc.tile_pool(name="pIn", bufs=2))
    ps_pre = ctx.enter_context(tc.tile_pool(name="ps_pre", bufs=2, space="PSUM"))
    ps_p1 = ctx.enter_context(tc.tile_pool(name="ps_p1", bufs=2, space="PSUM"))
    ps_p2 = ctx.enter_context(tc.tile_pool(name="ps_p2", bufs=2, space="PSUM"))
    ps_misc = ctx.enter_context(tc.tile_pool(name="ps_misc", bufs=2, space="PSUM"))

    def mps(name, dt=F32):
        return ps_misc.tile([128, 512], dt, name=name, tag="mps")

    # ---- engine load balancer for PSUM->SBUF relu/copy traffic (ns units)
    eng_load = {"act": 0.0, "dve": 0.0, "pool": 0.0}
    ECOST = {"act": lambda w: w * 0.833 + 170.0,
             "dve": lambda w: w * 1.0417 + 125.0,
             "pool": lambda w: w * 1.39 + 131.0}

    def assign(w, only=None, engines=("act", "dve")):
        # GPSIMD/Pool cannot access PSUM on TRN2, so PSUM-draining work is
        # restricted to ACT/DVE; SBUF-only ops may pass engines with "pool"
        if only is not None:
            e = only
        else:
            e = min(engines, key=lambda k: eng_load[k] + ECOST[k](w))
        eng_load[e] += ECOST[e](w)
        return e

    def emit_relu(out, in_, w, bias, only=None):
        e = assign(w, only)
        if e == "act":
            nc.scalar.activation(out, in_, AF.Relu, bias=bias)
        elif e == "dve":
            nc.vector.tensor_scalar(out, in_, bias, 0.0,
                                    op0=ALU.add, op1=ALU.max)
        else:
            nc.gpsimd.tensor_scalar(out, in_, bias, 0.0,
                                    op0=ALU.add, op1=ALU.max)

    def emit_copy(out, in_, w, only=None):
        e = assign(w, only)
        if e == "act":
            nc.scalar.activation(out, in_, AF.Copy)
        elif e == "dve":
            nc.vector.tensor_copy(out, in_)
        else:
            nc.gpsimd.tensor_copy(out, in_)

    def charge_ns(e, ns):
        eng_load[e] += ns

    # ---- resident loads (small/early-needed tensors first)
    bc_sb = pW.tile([H, 3 * L + 3], F32, name="bc_sb")
    nc.sync.dma_start(bc_sb[:], d["d_bc"].ap())
    wcols = sum(w for (_, w) in wmap.values())
    wblob_sb = pW.tile([128, wcols], BF16, name="wblob_sb")
    nc.sync.dma_start(wblob_sb[:], d["d_wblob"].ap())
    atomb_sb = pW.tile([AV, 9 * 2 * H], FP8, name="atomb_sb")
    nc.sync.dma_start(atomb_sb[:], d["d_atomb"].ap())
    bw2_sb = pW.tile([24, L * 2 * H], FP8, name="bw2_sb")
    nc.sync.dma_start(bw2_sb[:], d["d_bw2"].ap())
    bh2_sb = pW.tile([OUT, 1], F32, name="bh2_sb")
    nc.sync.dma_start(bh2_sb[:], d["d_bh2"].ap())
    F32R = mybir.dt.float32r
    whb_sb = pW.tile([128, 2 * H + 1], F32R, name="whb_sb")
    nc.sync.dma_start(whb_sb[:], d["d_whb"].ap())

    def W(name):
        off, w = wmap[name]
        return wblob_sb[:, off:off + w]

    idn_sb = W("idn")

    if has_empty:
        mrow_sb = pW.tile([1, BL * N], F32, name="mrow_sb")
        nc.sync.dma_start(mrow_sb[:], d["d_mask"].ap())
        nrow_sb = pW.tile([1, BL * N], F32, name="nrow_sb")
        nc.sync.dma_start(nrow_sb[:], d["d_neg"].ap())
        mask_bc = pW.tile([128, BL * N], F32, name="mask_bc")
        nc.gpsimd.partition_broadcast(mask_bc[:], mrow_sb[:])
        neg_bc = pW.tile([128, BL * N], F32, name="neg_bc")
        nc.gpsimd.partition_broadcast(neg_bc[:], nrow_sb[:])

    gsd_sb = pG.tile([128, 2 * S_core], FP8, name="gsd_sb")
    soh2_sb = pG.tile([24, 2 * S_core], FP8, name="soh2_sb")

    # ---- node features via fp8 DoubleRow (hi,res) pairs; hidden0 = 0
    xoh_sb = axoh_sb[:, 9 * 2 * H:]
    nf_ps = mps("nf_ps")
    nf = pNM.tile([128, BL * N], BF16, name="nf")
    for half in range(2):
        # half-major order with a split copy: the first m1/m2 block only
        # needs graph 0's node columns, so layer 0 starts ~2us earlier
        for c in range(9):
            st = atomb_sb[:, c * 2 * H:(c + 1) * 2 * H].rearrange(
                "p (two m) -> p two m", two=2)
            co = c * BL * N + half * 256
            mv = xoh_sb[:, co:co + 256].rearrange(
                "p (one n) -> p one n", one=1).broadcast_to([AV, 2, 256])
            nc.tensor.matmul(nf_ps[:, half * 256:(half + 1) * 256], st, mv,
                             start=(c == 0), stop=(c == 8), perf_mode=DR)
        nc.scalar.activation(nf[:, half * 256:(half + 1) * 256],
                             nf_ps[:, half * 256:(half + 1) * 256],
                             AF.Copy, scale=1.0 / 16.0)
    hid0 = pNM.tile([128, BL * N], BF16, name="hid0", tag="hid0")
    nc.vector.memset(hid0[:], 0.0)
    # prime the ACT Sqrt table while the engine is idle (a mid-run table
    # load costs 1283ns on the saturated ACT engine)
    warm = pLN.tile([128, 1], F32, name="warm", tag="warm")
    nc.scalar.activation(warm[:], bc_sb[:, 3 * L:3 * L + 1], AF.Sqrt)

    for gg in range(BL):
        sl = slice(2 * gg * S_graph, 2 * (gg + 1) * S_graph)
        nc.sync.dma_start(gsd_sb[:, sl], d["d_gsd"].ap()[:, sl])
        nc.sync.dma_start(soh2_sb[:, sl], d["d_soh2"].ap()[:, sl])

    hid_prev = hid0

    # ----------------------------------------------------------------
    # Globally software-pipelined job loop: jobs = (layer, graph, tile).
    # Stage lags keep the in-order PE stream busy: at step i the PE runs
    # DR(i), p1(i-1), p2(i-2) while ACT/DVE/Pool run the relus of earlier
    # jobs, so no engine ever waits head-of-line on a cross-engine hop.
    # ----------------------------------------------------------------
    jobs = [(l, gg, t) for l in range(L) for gg in range(BL)
            for t in range(n_tiles)]
    J = len(jobs)
    st8 = {}          # per-job state: pre/p1/p2/msgs tiles
    layer_st = {}     # per-layer tiles (msgs_max, h_fm, hid)
    graph_st = {}     # per-(layer,graph) tiles (mt, hn, ...)

    def layer_state(l):
        if l in layer_st:
            return layer_st[l]
        s = dict(
            msgs_max=pLN.tile([128, BL * N], BF16, name=f"msgs_max{l}",
                              tag=f"msgs_max{l % 2}", bufs=1),
            h_fm=pLN.tile([128, BL * N], BF16, name=f"h_fm{l}",
                          tag=f"h_fm{l % 2}", bufs=1),
            hid=(pNM.tile([128, BL * N], BF16, name=f"hid{l + 1}",
                          tag=f"hid{(l + 1) % 2}") if l < L - 1 or True
                 else None),
            bw2_l=bw2_sb[:, l * 2 * H:(l + 1) * 2 * H].rearrange(
                "p (two m) -> p two m", two=2),
        )
        layer_st[l] = s
        return s

    def emit_m12a(l, gg):
        ls = layer_state(l)
        hid_in = hid0 if l == 0 else layer_st[l - 1]["hid"]
        gsl = slice(gg * N, (gg + 1) * N)
        ps_m = mps("ps_m")
        nc.tensor.matmul(ps_m[:, 0:2 * H], nf[:, gsl],
                         W(f"m12_{l}_0"), start=True, stop=(l == 0))
        if l > 0:
            nc.tensor.matmul(ps_m[:, 0:2 * H], hid_in[:, gsl],
                             W(f"m12_{l}_1"), start=False, stop=True)
        # optional hi/res two-term fp8 split of (m1|m2) (~bf16 accuracy)
        mt = pMB.tile([128, 4 * H], FP8, name=f"mt{gg}", tag=f"mt{gg}",
                      bufs=2)
        emit_copy(mt[:, 0:2 * H], ps_m[:, 0:2 * H], 2 * H)
        graph_st[(l, gg)] = dict(
            ps_m=ps_m, mt=mt,
            mt_hi=mt[:, 0:2 * H].rearrange("p (two m) -> p two m", two=2),
            mt_res=mt[:, 2 * H:4 * H].rearrange("p (two m) -> p two m",
                                                two=2))

    def emit_m12b(l, gg):
        if not MT_HIRES:
            return
        gs = graph_st[(l, gg)]
        mt, ps_m = gs["mt"], gs["ps_m"]
        nc.vector.tensor_tensor(mt[:, 2 * H:4 * H], ps_m[:, 0:2 * H],
                                mt[:, 0:2 * H], op=ALU.subtract)
        charge_ns("dve", 2 * H * 1.0417 + 125.0)

    def emit_dr(i):
        l, gg, t = jobs[i]
        gs = graph_st[(l, gg)]
        ls = layer_state(l)
        w = tile_w[t]
        base = 2 * (gg * S_graph + t * 512)
        pre = ps_pre.tile([128, 512], F32, name="pre")
        nchunks = 1 if w <= 256 else 2
        chunks = []
        for c in range(nchunks):
            wc = min(w, 256) if c == 0 else w - 256
            chunks.append((slice(c * 256, c * 256 + wc), base + c * 512, wc))
        # grouped by stationary operand to minimize LdWeights swaps; one
        # accumulation group per PSUM bank (start arms the whole 2KB
        # region, chunk 1's first write replaces, later terms accumulate)
        stns = (gs["mt_hi"], gs["mt_res"]) if MT_HIRES else (gs["mt_hi"],)
        for si, stn in enumerate(stns):
            for ci, (cr, go, wc) in enumerate(chunks):
                mv_g = gsd_sb[:, go:go + 2 * wc].rearrange(
                    "p (two n) -> p two n", two=2)
                nc.tensor.matmul(pre[:, cr], stn, mv_g,
                                 start=(si == 0 and ci == 0),
                                 stop=False, perf_mode=DR)
        for ci, (cr, go, wc) in enumerate(chunks):
            mv_s = soh2_sb[:, go:go + 2 * wc].rearrange(
                "p (two n) -> p two n", two=2)
            nc.tensor.matmul(pre[:, cr], ls["bw2_l"], mv_s,
                             start=False, stop=(ci == len(chunks) - 1),
                             perf_mode=DR)
        msgs1 = pAct.tile([128, 512], BF16, name="msgs1", tag="msgs1")
        emit_relu(msgs1[:, 0:w], pre[:, 0:w], w, bc_sb[:, l:l + 1])
        st8[i] = dict(msgs1=msgs1)

    def emit_p1(i):
        l, gg, t = jobs[i]
        w = tile_w[t]
        p1 = ps_p1.tile([128, 512], F32, name="p1")
        nc.tensor.matmul(p1[:, 0:w], W(f"Wp1_{l}"), st8[i]["msgs1"][:, 0:w],
                         start=True, stop=True)
        msgs2 = pAct.tile([128, 512], BF16, name="msgs2", tag="msgs2")
        emit_relu(msgs2[:, 0:w], p1[:, 0:w], w, bc_sb[:, L + l:L + l + 1])
        st8[i]["msgs2"] = msgs2

    def emit_p2(i):
        l, gg, t = jobs[i]
        ls = layer_st[l]
        w = tile_w[t]
        p2 = ps_p2.tile([128, 512], F32, name="p2")
        nc.tensor.matmul(p2[:, 0:w], W(f"Wp2_{l}"), st8[i]["msgs2"][:, 0:w],
                         start=True, stop=True)
        (p0, R, K) = tiles[t]
        nc.vector.tensor_reduce(
            ls["msgs_max"][:, gg * N + p0: gg * N + p0 + R],
            p2[:, 0:w].rearrange("p (r k) -> p r k", r=R),
            axis=AX.X, op=ALU.max)
        charge_ns("dve", w * 1.0417 + 125.0)
        del st8[i]

    def emit_h(l, gg):
        ls = layer_st[l]
        hid_in = hid0 if l == 0 else layer_st[l - 1]["hid"]
        gsl = slice(gg * N, (gg + 1) * N)
        msl = ls["msgs_max"][:, gsl]
        if has_empty:
            mm1 = pLN.tile([128, N], F32, name="mm1", tag="mm1")
            nc.vector.tensor_tensor(mm1[:], msl, mask_bc[:, gsl],
                                    op=ALU.mult)
            nc.vector.tensor_tensor(mm1[:], mm1[:], neg_bc[:, gsl],
                                    op=ALU.add)
            mm2 = pLN.tile([128, N], BF16, name="mm2", tag="mm2")
            nc.vector.tensor_copy(mm2[:], mm1[:])
            msl = mm2[:]
        h_ps = mps("h_ps")
        nc.tensor.matmul(h_ps[:, 0:N], W(f"Wo1_{l}_0"), nf[:, gsl],
                         start=True, stop=False)
        nc.tensor.matmul(h_ps[:, 0:N], W(f"Wo1_{l}_1"), hid_in[:, gsl],
                         start=False, stop=False)
        nc.tensor.matmul(h_ps[:, 0:N], W(f"Wo2_{l}"), msl,
                         start=False, stop=True)
        emit_relu(ls["h_fm"][:, gsl], h_ps[:, 0:N], N,
                  bc_sb[:, 2 * L + l:2 * L + l + 1])

    def emit_ln1a(l, gg):
        # transpose to node-major; bn_stats reads the PSUM tile directly
        ls = layer_st[l]
        gsl = slice(gg * N, (gg + 1) * N)
        tp = mps("tp_ps", BF16)
        nc.tensor.transpose(tp[:, 0:128], ls["h_fm"][:, gsl], idn_sb)
        bn6 = pLN.tile([128, 6], F32, name="bn6", tag=f"bn6_{gg}", bufs=1)
        nc.vector.bn_stats(bn6[:], tp[:, 0:128])
        charge_ns("dve", 260.0)
        if l == L - 1:
            # drain phase: free the PSUM ring slot fast so the remaining
            # ladders don't serialize behind each other
            hT = pLN.tile([128, 128], BF16, name="hT", tag=f"hT_{gg}",
                          bufs=1)
            emit_copy(hT[:], tp[:, 0:128], 128)
            graph_st[(l, gg)].update(tp=None, hT=hT, bn6=bn6)
        else:
            graph_st[(l, gg)].update(tp=tp, hT=None, bn6=bn6)

    def emit_ln1b(l, gg):
        gs = graph_st[(l, gg)]
        bn2 = pLN.tile([128, 2], F32, name="bn2", tag=f"bn2_{gg}", bufs=1)
        nc.vector.bn_aggr(bn2[:], gs["bn6"][:])
        std = pLN.tile([128, 1], F32, name="std", tag=f"std_{gg}", bufs=1)
        nc.scalar.activation(std[:], bn2[:, 1:2], AF.Sqrt,
                             bias=bc_sb[:, 3 * L:3 * L + 1])
        gs.update(bn2=bn2, std=std)

    def emit_ln1c(l, gg):
        # normalize on DVE straight from the bf16 PSUM transpose (2-byte
        # packed operands hit the 2x DVE mode); rstd via DVE reciprocal
        gs = graph_st[(l, gg)]
        rstd = pLN.tile([128, 1], F32, name="rstd", tag=f"rstd_{gg}",
                        bufs=1)
        nc.vector.reciprocal(rstd[:], gs["std"][:])
        hn = pLN.tile([128, 128], BF16, name="hn", tag=f"hn_{gg}", bufs=1)
        src_ap = gs["hT"][:] if gs["tp"] is None else gs["tp"][:, 0:128]
        nc.vector.tensor_scalar(hn[:], src_ap, gs["bn2"][:, 0:1],
                                rstd[:], op0=ALU.subtract, op1=ALU.mult)
        charge_ns("dve", 192.0)
        gs["hn"] = hn

    def emit_ln2(l, gg):
        # transpose back to feature-major hid
        ls = layer_st[l]
        gsl = slice(gg * N, (gg + 1) * N)
        tp2 = mps("tp2_ps", BF16)
        nc.tensor.transpose(tp2[:, 0:128], graph_st[(l, gg)]["hn"][:],
                            idn_sb)
        emit_copy(ls["hid"][:, gsl], tp2[:, 0:128], N)

    LAST = n_tiles - 1
    LAGP1, LAGP2 = 2, 3
    import collections as _c
    sched = _c.defaultdict(list)
    for j, (l, gg, t) in enumerate(jobs):
        if t == 0:
            sched[max(j - 5, 0)].append(("m12a", l, gg))
            sched[max(j - 4, 0)].append(("m12b", l, gg))
        if t == LAST:
            e = j
            sched[e + LAGP2].append(("h", l, gg))
            sched[e + LAGP2 + 2].append(("ln1a", l, gg))
            sched[e + LAGP2 + 3].append(("ln1b", l, gg))
            sched[e + LAGP2 + 4].append(("ln1c", l, gg))
            if l < L - 1:
                # the last layer is pooled straight from node-major hn
                sched[e + LAGP2 + 5].append(("ln2", l, gg))
    FN = dict(m12a=emit_m12a, m12b=emit_m12b, h=emit_h,
              ln1a=emit_ln1a, ln1b=emit_ln1b, ln1c=emit_ln1c, ln2=emit_ln2)
    TOT = max(sched.keys()) + 1 if sched else 0
    TOT = max(TOT, J + LAGP2 + 1)
    for i in range(TOT):
        acts = sched.pop(i, [])
        for a in acts:                      # m12 prefetches first
            if a[0].startswith("m12"):
                FN[a[0]](a[1], a[2])
        # p1/p2 before DR: frees their PSUM ring slots in PE program order
        if LAGP1 <= i < J + LAGP1:
            emit_p1(i - LAGP1)
        if LAGP2 <= i < J + LAGP2:
            emit_p2(i - LAGP2)
        for a in acts:                      # h/LN stages after p2+reduce
            if not a[0].startswith("m12"):
                FN[a[0]](a[1], a[2])
        if i < J:
            emit_dr(i)
    hid_prev = layer_st[L - 1]["hid"]

    # ---- pooling + prediction MLP: graph_emb[f] = mean_n hid[n, f]
    # pooled directly from node-major hn via ones-column matmuls
    ge_ps = mps("ge_ps")
    for gg in range(BL):
        nc.tensor.matmul(ge_ps[:, gg:gg + 1], graph_st[(L - 1, gg)]["hn"][:],
                         W("ones"), start=True, stop=True)
    ge = pLN.tile([128, BL], F32R, name="ge", tag="ge")
    nc.scalar.activation(ge[:], ge_ps[:, 0:BL], AF.Copy, scale=1.0 / N)
    o1 = mps("o1_ps")
    nc.tensor.matmul(o1[:, 0:BL], whb_sb[:, 0:H], ge[:], start=True, stop=True)
    t1 = pLN.tile([128, BL], F32R, name="t1", tag="t1")
    nc.scalar.activation(t1[:], o1[:, 0:BL], AF.Relu,
                         bias=bc_sb[:, 3 * L + 1:3 * L + 2])
    o2 = mps("o2_ps")
    nc.tensor.matmul(o2[:, 0:BL], whb_sb[:, H:2 * H], t1[:],
                     start=True, stop=True)
    out_sb = pLN.tile([OUT, BL], F32, name="out_sb", tag="out_sb")
    nc.scalar.activation(out_sb[:], o2[:, 0:BL], AF.Identity,
                         bias=bh2_sb[:])
    nc.sync.dma_start(d["d_out"].ap(), out_sb[:])
    ctx.close()


# --------------------------------------------------------------------------
# Entry point.
# --------------------------------------------------------------------------

def build(inputs):
    struct, percore = _prep(inputs)
    A = _weight_arrays(inputs)
    wmap = A.pop("_wmap")
    key = (struct["S_graph"], struct["tiles"], struct["has_empty"])
    if key not in _CACHE:
        _CACHE[key] = _build_program(struct, wmap, A["wblob"].shape[1])
    nc = _CACHE[key]

    in_maps = []
    atomb = A.pop("atomb")
    for c in range(M):
        im = dict(
            gsd=percore["gsd"][c], soh2=percore["soh2"][c],
            axoh=np.ascontiguousarray(
                np.concatenate([atomb, percore["xoh"][c]], 1)),
            maskrow=percore["maskrow"][c:c + 1],
            negrow=percore["negrow"][c:c + 1],
        )
        for k, v in A.items():
            im[k] = v
        in_maps.append(im)
    return nc, in_maps, struct


def kernel(**inputs):
    from concourse import bass_utils
    nc, in_maps, struct = build(inputs)
    res = bass_utils.run_bass_kernel_spmd(nc, in_maps, core_ids=list(range(M)))
    out = np.zeros((B, OUT), np.float32)
    for c in range(M):
        out[c * BL:(c + 1) * BL] = res.results[c]["out"].T
    return out
